# revision 5
# baseline (speedup 1.0000x reference)
"""Trainium2 Bass kernel for nn_DHCSTGCN (TCN encoder + GRU rounds + fusion).

Math note: in the reference, the confidence-modulation / attention block only
reaches the output through att.sum(-1), which is the sum of a softmax == 1
(up to fp32 rounding ~1e-6).  `messages` therefore equals the msg-linear
output `v` exactly, and the whole [B,N,H/2] block (and g/m/c/r_vec inputs)
drops out of the computation.

Sharding: data-parallel over batch, 64 rows per core on 8 cores; all weights
replicated.  Outputs are gathered/concatenated on host.

Layout: the TCN stage runs in a padded row space — each batch occupies 32
rows (30 time steps + 2 zero rows), so a 4-batch chunk is exactly 128 rows.
The zero columns double as conv SAME-padding when the transposed activations
are read with a +-1 shifted stride-1 slice, keeping every matmul stationary
operand a single free dimension.
"""

import numpy as np

import concourse.bacc as bacc
import concourse.bass as bass
import concourse.tile as tile
from concourse import mybir
from concourse.masks import make_identity
from concourse.bass_utils import run_bass_kernel_spmd

F32 = mybir.dt.float32
AF = mybir.ActivationFunctionType
OP = mybir.AluOpType
AX = mybir.AxisListType

B, N, H, W, P, RMAX = 512, 512, 256, 30, 7, 30
NCORES = 8
BC = B // NCORES      # 64 batch rows per core
WP = 32               # padded time steps per batch
CB = 4                # batches per row-chunk
RCH = CB * WP         # 128 padded rows per chunk
NCHUNK = BC // CB     # 16 chunks
H2 = H // 2           # 128
H3 = 3 * H            # 768
EPS = 1e-5


def _bcast(ap, parts):
    """Partition-broadcast a DRAM AP: [d...] -> [parts, d...] with step 0."""
    return bass.AP(tensor=ap.tensor, offset=ap.offset,
                   ap=[[0, parts]] + [list(d) for d in ap.ap])


def _ln_rows(nc, wk, src, dst, rows, gb, bb, eps_t):
    """LayerNorm over the free dim of src [rows, D] (src is clobbered)."""
    st = wk.tile([128, 6], F32, tag="lnst")
    nc.vector.bn_stats(out=st[:rows], in_=src)
    mv = wk.tile([128, 2], F32, tag="lnmv")
    nc.vector.bn_aggr(out=mv[:rows], in_=st[:rows])
    sd = wk.tile([128, 1], F32, tag="lnsd")
    nc.scalar.activation(out=sd[:rows], in_=mv[:rows, 1:2], func=AF.Sqrt,
                         bias=eps_t[:rows], scale=1.0)
    nc.vector.reciprocal(out=sd[:rows], in_=sd[:rows])
    nc.vector.tensor_scalar(out=src, in0=src, scalar1=mv[:rows, 0:1],
                            scalar2=sd[:rows], op0=OP.subtract, op1=OP.mult)
    nc.vector.tensor_mul(out=src, in0=src, in1=gb[:rows])
    nc.vector.tensor_add(out=dst, in0=src, in1=bb[:rows])


def _build(R):
    nc = bacc.Bacc("TRN2", target_bir_lowering=False, debug=False)

    di = lambda name, shape: nc.dram_tensor(name, shape, F32, kind="ExternalInput")
    do = lambda name, shape: nc.dram_tensor(name, shape, F32, kind="ExternalOutput")

    x4_d = di("x4", [4, BC, WP])
    blk_d = di("blk", [RCH, 8, 32])
    w1b_d = di("w1b", [4, H])
    ln1g_d = di("ln1g", [H]); ln1b_d = di("ln1b", [H])
    ln2g_d = di("ln2g", [H]); ln2b_d = di("ln2b", [H])
    w2f_d = di("w2f", [128, 6, H])
    b2row_d = di("b2row", [1, H])
    wih_d = di("wihT", [R, 128, 2, H3])
    whh_d = di("whhT", [R, 128, 2, H3])
    msgw_d = di("msgwT", [R, 128, 2, H])
    hw1_d = di("hw1T", [R, 128, 2, H2])
    hw2_d = di("hw2T", [R, H2, P])
    brz_d = di("brz", [R, 128, 4])
    bin_d = di("binb", [R, 128, 2])
    bhn_d = di("bhnb", [R, 128, 2])
    msgb_d = di("msgb", [R, 128, 2])
    hb1_d = di("hb1", [R, 128, 1])
    hb2_d = di("hb2", [R, P, 1])
    fw1a_d = di("fw1aT", [R * P, BC])
    fw1b_d = di("fw1bT", [128, 2, BC])
    fb1_d = di("fb1", [BC, 1])
    flng_d = di("flng", [BC]); flnb_d = di("flnb", [BC])
    fw2_d = di("fw2T", [BC, RMAX])
    fb2_d = di("fb2", [RMAX, 1])

    fin_d = do("final", [BC, P])
    pers_d = do("pers_act", [BC, R * P])
    fw_d = do("fw", [BC, RMAX])

    with tile.TileContext(nc) as tc:
        with (
            tc.tile_pool(name="singles", bufs=1) as sg,
            tc.tile_pool(name="wk", bufs=3) as wk,
            tc.tile_pool(name="wp", bufs=2) as wp,
            tc.tile_pool(name="rw", bufs=2) as rw,
            tc.tile_pool(name="psA", bufs=2, space="PSUM") as psA,
            tc.tile_pool(name="psB", bufs=2, space="PSUM") as psB,
            tc.tile_pool(name="psT", bufs=1, space="PSUM") as psT,
            tc.tile_pool(name="psM", bufs=1, space="PSUM") as psM,
            tc.tile_pool(name="psR", bufs=2, space="PSUM") as psR,
        ):
            # ---------- constants / replicated weights ----------
            ident = sg.tile([128, 128], F32, tag="ident")
            make_identity(nc, ident[:, :])

            eps_t = sg.tile([128, 1], F32, tag="eps")
            nc.vector.memset(eps_t[:, :], EPS)

            ones1 = sg.tile([1, 128], F32, tag="ones1")
            nc.vector.memset(ones1[:, :], 1.0)

            blk = sg.tile([RCH, 8, 32], F32, tag="blk")
            nc.sync.dma_start(out=blk[:, :, :], in_=blk_d[:, :, :])

            w1b_t = sg.tile([4, H], F32, tag="w1b")
            nc.sync.dma_start(out=w1b_t[:, :], in_=w1b_d[:, :])
            w2f_t = sg.tile([128, 6, H], F32, tag="w2f")
            nc.sync.dma_start(out=w2f_t[:, 0:3, :], in_=w2f_d[:, 0:3, :])
            nc.sync.dma_start(out=w2f_t[:, 3:6, :], in_=w2f_d[:, 3:6, :])
            b2row_t = sg.tile([1, H], F32, tag="b2row")
            nc.sync.dma_start(out=b2row_t[:, :], in_=b2row_d[:, :])

            g1b = sg.tile([128, H], F32, tag="g1b")
            nc.sync.dma_start(out=g1b[:, :], in_=_bcast(ln1g_d[:], 128))
            b1b = sg.tile([128, H], F32, tag="b1b")
            nc.sync.dma_start(out=b1b[:, :], in_=_bcast(ln1b_d[:], 128))
            g2b = sg.tile([128, H], F32, tag="g2b")
            nc.sync.dma_start(out=g2b[:, :], in_=_bcast(ln2g_d[:], 128))
            b2b = sg.tile([128, H], F32, tag="b2b")
            nc.sync.dma_start(out=b2b[:, :], in_=_bcast(ln2b_d[:], 128))

            flngb = sg.tile([BC, BC], F32, tag="flngb")
            nc.sync.dma_start(out=flngb[:, :], in_=_bcast(flng_d[:], BC))
            flnbb = sg.tile([BC, BC], F32, tag="flnbb")
            nc.sync.dma_start(out=flnbb[:, :], in_=_bcast(flnb_d[:], BC))
            fb1_t = sg.tile([BC, 1], F32, tag="fb1")
            nc.sync.dma_start(out=fb1_t[:, :], in_=fb1_d[:, :])
            fw2_t = sg.tile([BC, RMAX], F32, tag="fw2")
            nc.sync.dma_start(out=fw2_t[:, :], in_=fw2_d[:, :])
            fb2_t = sg.tile([RMAX, 1], F32, tag="fb2")
            nc.sync.dma_start(out=fb2_t[:, :], in_=fb2_d[:, :])
            fw1b_t = sg.tile([128, 2, BC], F32, tag="fw1b")
            nc.sync.dma_start(out=fw1b_t[:, :, :], in_=fw1b_d[:, :, :])
            fw1a_t = []
            for r in range(R):
                t = sg.tile([P, BC], F32, tag=f"fw1a{r}", name=f"fw1a{r}")
                nc.sync.dma_start(out=t[:, :], in_=fw1a_d[r * P:(r + 1) * P, :])
                fw1a_t.append(t)

            # ---------- conv1 im2col of x (host-prepared): [4, BC*WP] ----------
            t4 = sg.tile([4, BC * WP], F32, tag="t4")
            nc.sync.dma_start(out=t4[:, :],
                              in_=x4_d[:, :, :].rearrange("k b w -> k (b w)"))

            h_cur = sg.tile([BC, H], F32, tag="h_cur")

            # ---------- TCN: 16 chunks of 4 batches x 32 padded rows ----------
            for j in range(NCHUNK):
                ps1 = psA.tile([RCH, H], F32, tag="ps1")
                nc.tensor.matmul(ps1[:, :], t4[:, j * RCH:(j + 1) * RCH],
                                 w1b_t[:, :], start=True, stop=True)
                y = wk.tile([RCH, H], F32, tag="y")
                nc.scalar.activation(out=y[:, :], in_=ps1[:, :], func=AF.Relu)
                yln = wk.tile([RCH, H], F32, tag="yln")
                _ln_rows(nc, wk, y[:, :], yln[:, :], RCH, g1b, b1b, eps_t)

                # transpose yln -> yT [c, cc, 1+row'] ; pad cols stay zero
                yT = wk.tile([128, 2, RCH + 2], F32, tag="yT")
                nc.vector.memset(yT[:, :, :], 0.0)
                for cc in range(2):
                    pst = psT.tile([128, 128], F32, tag="pst")
                    nc.tensor.transpose(pst[:, :],
                                        yln[:, cc * 128:(cc + 1) * 128],
                                        ident[:, :])
                    nc.scalar.copy(
                        out=yT[:, cc, 1:RCH + 1]
                        .rearrange("p (b w) -> p b w", w=WP)[:, :, 0:W],
                        in_=pst[:, :]
                        .rearrange("p (b w) -> p b w", w=WP)[:, :, 0:W])

                # conv2 (+bias) as 6 shifted matmuls + 1 bias matmul
                ps2 = psB.tile([RCH, H], F32, tag="ps2")
                kk = 0
                for cc in range(2):
                    for dw in range(3):
                        nc.tensor.matmul(ps2[:, :], yT[:, cc, dw:dw + RCH],
                                         w2f_t[:, cc * 3 + dw, :],
                                         start=(kk == 0), stop=False)
                        kk += 1
                nc.tensor.matmul(ps2[:, :], ones1[:, :], b2row_t[:, :],
                                 start=False, stop=True)

                t2 = wk.tile([RCH, H], F32, tag="t2")
                nc.scalar.activation(out=t2[:, :], in_=ps2[:, :], func=AF.Relu)
                y2 = wk.tile([RCH, H], F32, tag="y2")
                nc.vector.tensor_add(out=y2[:, :], in0=t2[:, :], in1=yln[:, :])
                y2n = wk.tile([RCH, H], F32, tag="y2n")
                _ln_rows(nc, wk, y2[:, :], y2n[:, :], RCH, g2b, b2b, eps_t)

                gi, jj = divmod(j, 8)
                if jj == 0:
                    psm32 = psM.tile([32, H], F32, tag="psm", name="psm32")
                nc.tensor.matmul(psm32[:, :], blk[:, jj, :], y2n[:, :],
                                 start=(jj == 0), stop=(jj == 7))
                if jj == 7:
                    nc.scalar.mul(out=h_cur[gi * 32:(gi + 1) * 32, :],
                                  in_=psm32[:, :], mul=1.0 / W)

            # ---------- h_current transposed: hT [c, cc, b] ----------
            hT = sg.tile([128, 2, BC], F32, tag="hT")
            for cc in range(2):
                pst = psT.tile([128, 128], F32, tag="pst")
                nc.tensor.transpose(pst[:, :BC],
                                    h_cur[:, cc * 128:(cc + 1) * 128],
                                    ident[:BC, :BC])
                nc.scalar.copy(out=hT[:, cc, :], in_=pst[:, :BC])

            pred_t = [sg.tile([P, BC], F32, tag=f"pred{r}", name=f"pred{r}")
                      for r in range(R)]

            # ---------- rounds ----------
            hp2 = [hT[:, 0, :], hT[:, 1, :]]
            for r in range(R):
                wih = wp.tile([128, 2, H3], F32, tag="wih")
                for q in range(4):
                    s0, s1 = q * (H3 // 2), (q + 1) * (H3 // 2)
                    cc, lo, hi = (0, s0, s1) if q < 2 else (1, s0 - H3, s1 - H3)
                    nc.sync.dma_start(out=wih[:, cc, lo:hi],
                                      in_=wih_d[r, :, cc, lo:hi])
                whh = wp.tile([128, 2, H3], F32, tag="whh")
                for q in range(4):
                    s0, s1 = q * (H3 // 2), (q + 1) * (H3 // 2)
                    cc, lo, hi = (0, s0, s1) if q < 2 else (1, s0 - H3, s1 - H3)
                    nc.sync.dma_start(out=whh[:, cc, lo:hi],
                                      in_=whh_d[r, :, cc, lo:hi])
                msgw = wp.tile([128, 2, H], F32, tag="msgw")
                nc.sync.dma_start(out=msgw[:, 0, :], in_=msgw_d[r, :, 0, :])
                nc.sync.dma_start(out=msgw[:, 1, :], in_=msgw_d[r, :, 1, :])
                hw1 = wp.tile([128, 2, H2], F32, tag="hw1")
                nc.sync.dma_start(out=hw1[:, :, :], in_=hw1_d[r, :, :, :])
                hw2 = wp.tile([H2, P], F32, tag="hw2")
                nc.sync.dma_start(out=hw2[:, :], in_=hw2_d[r, :, :])
                brz = wp.tile([128, 4], F32, tag="brz")
                nc.sync.dma_start(out=brz[:, :], in_=brz_d[r, :, :])
                binb = wp.tile([128, 2], F32, tag="binb")
                nc.sync.dma_start(out=binb[:, :], in_=bin_d[r, :, :])
                bhnb = wp.tile([128, 2], F32, tag="bhnb")
                nc.sync.dma_start(out=bhnb[:, :], in_=bhn_d[r, :, :])
                msgb = wp.tile([128, 2], F32, tag="msgb")
                nc.sync.dma_start(out=msgb[:, :], in_=msgb_d[r, :, :])
                hb1 = wp.tile([128, 1], F32, tag="hb1")
                nc.sync.dma_start(out=hb1[:, :], in_=hb1_d[r, :, :])
                hb2 = wp.tile([P, 1], F32, tag="hb2")
                nc.sync.dma_start(out=hb2[:, :], in_=hb2_d[r, :, :])

                # messages = msg_w @ h_current + msg_b  (attention sum == 1)
                vs = []
                for m in range(2):
                    pv = psR.tile([128, BC], F32, tag="pr")
                    for cc in range(2):
                        nc.tensor.matmul(pv[:, :],
                                         msgw[:, cc, m * 128:(m + 1) * 128],
                                         hT[:, cc, :],
                                         start=(cc == 0), stop=(cc == 1))
                    v = rw.tile([128, BC], F32, tag=f"v{m}")
                    nc.scalar.activation(out=v[:, :], in_=pv[:, :],
                                         func=AF.Identity,
                                         bias=msgb[:, m:m + 1])
                    vs.append(v)

                # r/z gates: sigmoid(Wih@v + Whh@h + bih + bhh)
                gates = []
                for m in range(4):
                    pg = psR.tile([128, BC], F32, tag="pr")
                    kk = 0
                    for wt, xs in ((wih, vs), (whh, hp2)):
                        for cc in range(2):
                            nc.tensor.matmul(pg[:, :],
                                             wt[:, cc, m * 128:(m + 1) * 128],
                                             xs[cc],
                                             start=(kk == 0), stop=(kk == 3))
                            kk += 1
                    g = rw.tile([128, BC], F32, tag=f"g{m}")
                    nc.scalar.activation(out=g[:, :], in_=pg[:, :],
                                         func=AF.Sigmoid, bias=brz[:, m:m + 1])
                    gates.append(g)

                # n gate: tanh(Wih_n@v + bin + rg * (Whh_n@h + bhn))
                ns = []
                for mi in range(2):
                    m = 4 + mi
                    pi = psR.tile([128, BC], F32, tag="pr")
                    for cc in range(2):
                        nc.tensor.matmul(pi[:, :],
                                         wih[:, cc, m * 128:(m + 1) * 128],
                                         vs[cc][:, :],
                                         start=(cc == 0), stop=(cc == 1))
                    ph = psR.tile([128, BC], F32, tag="pr")
                    for cc in range(2):
                        nc.tensor.matmul(ph[:, :],
                                         whh[:, cc, m * 128:(m + 1) * 128],
                                         hp2[cc],
                                         start=(cc == 0), stop=(cc == 1))
                    hn = rw.tile([128, BC], F32, tag=f"hn{mi}")
                    nc.scalar.activation(out=hn[:, :], in_=ph[:, :],
                                         func=AF.Identity,
                                         bias=bhnb[:, mi:mi + 1])
                    nc.vector.tensor_mul(out=hn[:, :], in0=gates[mi][:, :],
                                         in1=hn[:, :])
                    nc.vector.tensor_add(out=hn[:, :], in0=hn[:, :],
                                         in1=pi[:, :])
                    n_t = rw.tile([128, BC], F32, tag=f"n{mi}")
                    nc.scalar.activation(out=n_t[:, :], in_=hn[:, :],
                                         func=AF.Tanh, bias=binb[:, mi:mi + 1])
                    ns.append(n_t)

                # h' = n + z*(h - n)
                hnew = rw.tile([128, 2, BC], F32, tag="hstate")
                for mi in range(2):
                    d = rw.tile([128, BC], F32, tag=f"d{mi}")
                    nc.vector.tensor_sub(out=d[:, :], in0=hp2[mi],
                                         in1=ns[mi][:, :])
                    nc.vector.tensor_mul(out=d[:, :], in0=gates[2 + mi][:, :],
                                         in1=d[:, :])
                    nc.vector.tensor_add(out=hnew[:, mi, :], in0=ns[mi][:, :],
                                         in1=d[:, :])
                hp2 = [hnew[:, 0, :], hnew[:, 1, :]]

                # head: pred = head_w2 @ relu(head_w1 @ h' + b1) + b2
                pp = psR.tile([128, BC], F32, tag="pr")
                for cc in range(2):
                    nc.tensor.matmul(pp[:H2, :], hw1[:, cc, :], hp2[cc],
                                     start=(cc == 0), stop=(cc == 1))
                p1 = rw.tile([H2, BC], F32, tag="p1")
                nc.scalar.activation(out=p1[:, :], in_=pp[:H2, :],
                                     func=AF.Relu, bias=hb1[:H2, 0:1])
                pq = psR.tile([128, BC], F32, tag="pr")
                nc.tensor.matmul(pq[:P, :], hw2[:, :], p1[:, :],
                                 start=True, stop=True)
                nc.scalar.activation(out=pred_t[r][:, :], in_=pq[:P, :],
                                     func=AF.Identity, bias=hb2[:, 0:1])

            # ---------- fusion ----------
            phf = psR.tile([128, BC], F32, tag="pr")
            nk = R + 2
            kk = 0
            for r in range(R):
                nc.tensor.matmul(phf[:BC, :], fw1a_t[r][:, :],
                                 pred_t[r][:, :],
                                 start=(kk == 0), stop=False)
                kk += 1
            for cc in range(2):
                nc.tensor.matmul(phf[:BC, :], fw1b_t[:, cc, :], hp2[cc],
                                 start=(kk == 0), stop=(kk == nk - 1))
                kk += 1
            hfT = rw.tile([BC, BC], F32, tag="hfT")
            nc.scalar.activation(out=hfT[:, :], in_=phf[:BC, :],
                                 func=AF.Relu, bias=fb1_t[:, 0:1])
            pstf = psT.tile([128, 128], F32, tag="pst")
            nc.tensor.transpose(pstf[:BC, :BC], hfT[:, :], ident[:BC, :BC])
            hfr = rw.tile([BC, BC], F32, tag="hfr")
            nc.scalar.copy(out=hfr[:, :], in_=pstf[:BC, :BC])
            hfn = rw.tile([BC, BC], F32, tag="hfn")
            _ln_rows(nc, wk, hfr[:, :], hfn[:, :], BC, flngb, flnbb, eps_t)
            pstg = psT.tile([128, 128], F32, tag="pst")
            nc.tensor.transpose(pstg[:BC, :BC], hfn[:, :], ident[:BC, :BC])
            hfnT = rw.tile([BC, BC], F32, tag="hfnT")
            nc.scalar.copy(out=hfnT[:, :], in_=pstg[:BC, :BC])

            plg = psR.tile([128, BC], F32, tag="pr")
            nc.tensor.matmul(plg[:RMAX, :], fw2_t[:, :], hfnT[:, :],
                             start=True, stop=True)
            lgT = rw.tile([RMAX, BC], F32, tag="lgT")
            nc.scalar.activation(out=lgT[:, :], in_=plg[:RMAX, :],
                                 func=AF.Identity, bias=fb2_t[:, 0:1])
            psth = psT.tile([128, 128], F32, tag="pst")
            nc.tensor.transpose(psth[:BC, :RMAX], lgT[:, :],
                                ident[:RMAX, :RMAX])
            lg = rw.tile([BC, RMAX], F32, tag="lg")
            nc.scalar.copy(out=lg[:, :], in_=psth[:BC, :RMAX])

            mx = rw.tile([BC, 1], F32, tag="mx")
            nc.vector.reduce_max(out=mx[:, :], in_=lg[:, :], axis=AX.X)
            nc.vector.tensor_scalar(out=lg[:, :], in0=lg[:, :],
                                    scalar1=mx[:, 0:1], scalar2=None,
                                    op0=OP.subtract)
            ex = rw.tile([BC, RMAX], F32, tag="ex")
            sm = rw.tile([BC, 1], F32, tag="sm")
            nc.scalar.activation(out=ex[:, :], in_=lg[:, :], func=AF.Exp,
                                 accum_out=sm[:, 0:1])
            nc.vector.reciprocal(out=sm[:, :], in_=sm[:, :])
            fwr = rw.tile([BC, RMAX], F32, tag="fwr")
            nc.vector.tensor_scalar_mul(out=fwr[:, :], in0=ex[:, :],
                                        scalar1=sm[:, 0:1])
            nc.sync.dma_start(out=fw_d[:, :], in_=fwr[:, :])

            prs = rw.tile([BC, R * P], F32, tag="prs")
            for r in range(R):
                pstp = psT.tile([128, 128], F32, tag="pst")
                nc.tensor.transpose(pstp[:BC, :P], pred_t[r][:, :],
                                    ident[:P, :P])
                nc.scalar.copy(out=prs[:, r * P:(r + 1) * P],
                               in_=pstp[:BC, :P])
            nc.sync.dma_start(out=pers_d[:, :], in_=prs[:, :])

            fin = rw.tile([BC, P], F32, tag="fin")
            ftmp = rw.tile([BC, P], F32, tag="ftmp")
            for r in range(R):
                dst = fin if r == 0 else ftmp
                nc.vector.tensor_scalar_mul(out=dst[:, :],
                                            in0=prs[:, r * P:(r + 1) * P],
                                            scalar1=fwr[:, r:r + 1])
                if r > 0:
                    nc.vector.tensor_add(out=fin[:, :], in0=fin[:, :],
                                         in1=ftmp[:, :])
            nc.sync.dma_start(out=fin_d[:, :], in_=fin[:, :])

    nc.compile()
    return nc


_COMPILED = {}


def _get_compiled(R):
    if R not in _COMPILED:
        _COMPILED[R] = _build(R)
    return _COMPILED[R]


_BLK = np.zeros((RCH, 8, 32), np.float32)
for _p in range(RCH):
    if _p % WP < W:
        for _j in range(8):
            _BLK[_p, _j, 4 * _j + _p // WP] = 1.0


def _prep_x4(xs):
    x4 = np.zeros((4, BC, WP), np.float32)
    x4[0, :, 1:W] = xs[:, :W - 1]
    x4[1, :, :W] = xs
    x4[2, :, :W - 1] = xs[:, 1:]
    x4[3, :, :W] = 1.0
    return x4


def _prep_weights(inp, R):
    f = lambda a: np.ascontiguousarray(np.asarray(a, dtype=np.float32))
    conv1_w = f(inp["conv1_w"]); conv1_b = f(inp["conv1_b"])
    conv2_w = f(inp["conv2_w"]); conv2_b = f(inp["conv2_b"])
    w1b = np.zeros((4, H), np.float32)
    w1b[0:3] = conv1_w[:, 0, :].T
    w1b[3] = conv1_b
    w2f = conv2_w.transpose(1, 2, 0).reshape(2, 128, 3, H) \
        .transpose(1, 0, 2, 3).reshape(128, 6, H)
    gw = lambda a: f(a)[:R].transpose(0, 2, 1).reshape(R, 2, 128, -1) \
        .transpose(0, 2, 1, 3)
    bih = f(inp["gru_bih"])[:R]; bhh = f(inp["gru_bhh"])[:R]
    ch = lambda a: a.reshape(R, 2, 128).transpose(0, 2, 1)
    fusion_w1 = f(inp["fusion_w1"])
    m = {
        "w1b": w1b,
        "ln1g": f(inp["ln1_g"]), "ln1b": f(inp["ln1_b"]),
        "ln2g": f(inp["ln2_g"]), "ln2b": f(inp["ln2_b"]),
        "w2f": np.ascontiguousarray(w2f),
        "b2row": conv2_b[None, :],
        "wihT": np.ascontiguousarray(gw(inp["gru_wih"])),
        "whhT": np.ascontiguousarray(gw(inp["gru_whh"])),
        "msgwT": np.ascontiguousarray(gw(inp["msg_w"])),
        "hw1T": np.ascontiguousarray(gw(inp["head_w1"])),
        "hw2T": np.ascontiguousarray(f(inp["head_w2"])[:R].transpose(0, 2, 1)),
        "brz": np.ascontiguousarray(
            (bih + bhh)[:, :2 * H].reshape(R, 4, 128).transpose(0, 2, 1)),
        "binb": np.ascontiguousarray(ch(bih[:, 2 * H:])),
        "bhnb": np.ascontiguousarray(ch(bhh[:, 2 * H:])),
        "msgb": np.ascontiguousarray(ch(f(inp["msg_b"])[:R])),
        "hb1": np.ascontiguousarray(f(inp["head_b1"])[:R][:, :, None]),
        "hb2": np.ascontiguousarray(f(inp["head_b2"])[:R][:, :, None]),
        "fw1aT": np.ascontiguousarray(fusion_w1[:, :R * P].T),
        "fw1bT": np.ascontiguousarray(
            fusion_w1[:, RMAX * P:].T.reshape(2, 128, BC).transpose(1, 0, 2)),
        "fb1": f(inp["fusion_b1"])[:, None],
        "flng": f(inp["fusion_ln_g"]), "flnb": f(inp["fusion_ln_b"]),
        "fw2T": np.ascontiguousarray(f(inp["fusion_w2"]).T),
        "fb2": f(inp["fusion_b2"])[:, None],
        "blk": _BLK,
    }
    return m


def run_on_device(inputs, trace=False):
    """Shard, run the bass kernel on 8 cores, gather. Returns (outs, bkr)."""
    R = int(np.asarray(inputs["R"]))
    nc = _get_compiled(R)
    shared = _prep_weights(inputs, R)
    x = np.ascontiguousarray(np.asarray(inputs["x"], dtype=np.float32))
    in_maps = []
    for i in range(NCORES):
        mm = dict(shared)
        mm["x4"] = _prep_x4(x[i * BC:(i + 1) * BC])
        in_maps.append(mm)
    bkr = run_bass_kernel_spmd(nc, in_maps, core_ids=list(range(NCORES)),
                               trace=trace)
    res = bkr.results
    final = np.concatenate([res[i]["final"] for i in range(NCORES)], axis=0)
    pa = np.concatenate([res[i]["pers_act"] for i in range(NCORES)], axis=0)
    fw = np.concatenate([res[i]["fw"] for i in range(NCORES)], axis=0)
    pers = np.zeros((B, RMAX, P), np.float32)
    pers[:, :R, :] = pa.reshape(B, R, P)
    return (final.astype(np.float32), pers, fw.astype(np.float32)), bkr


def kernel(**inputs):
    outs, _ = run_on_device(inputs, trace=False)
    return outs


# revision 10
# speedup vs baseline: 2.4355x; 2.4355x over previous
"""Trainium2 Bass kernel for nn_DHCSTGCN (TCN encoder + GRU rounds + fusion).

Math note: in the reference, the confidence-modulation / attention block only
reaches the output through att.sum(-1), which is the sum of a softmax == 1
(up to fp32 rounding ~1e-6).  `messages` therefore equals the msg-linear
output `v` exactly, and the whole [B,N,H/2] block (and g/m/c/r_vec inputs)
drops out of the computation.

Sharding: data-parallel over batch, 64 rows per core on 8 cores; all weights
replicated.  Outputs are gathered/concatenated on host.

Layout: the TCN stage runs in a padded row space — each batch occupies 32
rows (30 time steps + 2 zero rows), so a 4-batch chunk is exactly 128 rows.
The zero columns double as conv SAME-padding when the transposed activations
are read with a +-1 shifted stride-1 slice, keeping every matmul stationary
operand a single free dimension.

Perf notes: big matmuls (moving dim 256) run as float32r (1 cycle/row vs 4
for fp32); the TCN is emitted phase-major so each engine's stream is dense;
LayerNorm gamma/beta of LN2 are folded into the h_current transpose-copy
(per-partition scale/bias on the ACT engine); LN1 gamma/beta and the
residual add run on the otherwise-idle GpSimd engine.
"""

import numpy as np

import concourse.bacc as bacc
import concourse.bass as bass
import concourse.tile as tile
from concourse import mybir
from concourse.masks import make_identity
from concourse.bass_utils import run_bass_kernel_spmd

F32 = mybir.dt.float32
F32R = mybir.dt.float32r
AF = mybir.ActivationFunctionType
OP = mybir.AluOpType
AX = mybir.AxisListType

B, N, H, W, P, RMAX = 512, 512, 256, 30, 7, 30
NCORES = 8
BC = B // NCORES      # 64 batch rows per core
WP = 32               # padded time steps per batch
CB = 4                # batches per row-chunk
RCH = CB * WP         # 128 padded rows per chunk
NCHUNK = BC // CB     # 16 chunks
H2 = H // 2           # 128
H3 = 3 * H            # 768
EPS = 1e-5


def _bcast(ap, parts):
    """Partition-broadcast a DRAM AP: [d...] -> [parts, d...] with step 0."""
    return bass.AP(tensor=ap.tensor, offset=ap.offset,
                   ap=[[0, parts]] + [list(d) for d in ap.ap])


def _r(ap):
    return ap.bitcast(F32R)


def _ln_core(nc, wk, src, dst, rows, eps_t):
    """y_hat = (src - mean)/sqrt(var+eps) over free dim (src clobbered)."""
    st = wk.tile([128, 6], F32, tag="lnst")
    nc.vector.bn_stats(out=st[:rows], in_=src)
    mv = wk.tile([128, 2], F32, tag="lnmv")
    nc.vector.bn_aggr(out=mv[:rows], in_=st[:rows])
    sd = wk.tile([128, 1], F32, tag="lnsd")
    nc.scalar.activation(out=sd[:rows], in_=mv[:rows, 1:2], func=AF.Sqrt,
                         bias=eps_t[:rows], scale=1.0)
    nc.vector.reciprocal(out=sd[:rows], in_=sd[:rows])
    nc.vector.tensor_scalar(out=dst, in0=src, scalar1=mv[:rows, 0:1],
                            scalar2=sd[:rows], op0=OP.subtract, op1=OP.mult)


def _build(R):
    nc = bacc.Bacc("TRN2", target_bir_lowering=False, debug=False)

    di = lambda name, shape: nc.dram_tensor(name, shape, F32, kind="ExternalInput")
    dir_ = lambda name, shape: nc.dram_tensor(name, shape, F32R, kind="ExternalInput")
    do = lambda name, shape: nc.dram_tensor(name, shape, F32, kind="ExternalOutput")

    x4_d = dir_("x4", [4, BC, WP])
    blk_d = dir_("blk", [RCH, 8, 32])
    w1b_d = dir_("w1b", [4, H])
    ln1g_d = di("ln1g", [H]); ln1b_d = di("b1bm", [128, H])
    g2c_d = di("g2c", [128, 2]); b2c_d = di("b2c", [128, 2])
    w2f_d = dir_("w2f", [128, 6, H])
    b2row_d = dir_("b2row", [1, H])
    wih_d = di("wihT", [R, 128, 2, H3])
    whh_d = di("whhT", [R, 128, 2, H3])
    msgw_d = di("msgwT", [R, 128, 2, H])
    hw1_d = di("hw1T", [R, 128, 2, H2])
    hw2_d = di("hw2T", [R, H2, P])
    ball_d = di("ball", [R, 128, 12])
    fw1a_d = di("fw1aT", [R * P, BC])
    fw1b_d = di("fw1bT", [128, 2, BC])
    fb1_d = di("fb1", [BC, 1])
    flng_d = di("flng", [BC]); flnb_d = di("flnb", [BC])
    fw2_d = di("fw2T", [BC, RMAX])
    fb2_d = di("fb2", [RMAX, 1])

    fin_d = do("final", [BC, P])
    pers_d = do("pers_act", [BC, R * P])
    fw_d = do("fw", [BC, RMAX])

    with tile.TileContext(nc) as tc:
        with (
            tc.tile_pool(name="singles", bufs=1) as sg,
            tc.tile_pool(name="wk", bufs=3) as wk,
            tc.tile_pool(name="wp", bufs=2) as wp,
            tc.tile_pool(name="rw", bufs=2) as rw,
            tc.tile_pool(name="psA", bufs=2, space="PSUM") as psA,
            tc.tile_pool(name="psB", bufs=3, space="PSUM") as psB,
            tc.tile_pool(name="psT", bufs=2, space="PSUM") as psT,
            tc.tile_pool(name="psM", bufs=1, space="PSUM") as psM,
        ):
            # ---------- critical-path inputs first ----------
            t4 = sg.tile([4, BC * WP], F32R, tag="t4")
            nc.sync.dma_start(out=t4[:, :],
                              in_=x4_d[:, :, :].rearrange("k b w -> k (b w)"))
            w1b_t = sg.tile([4, H], F32R, tag="w1b")
            nc.sync.dma_start(out=w1b_t[:, :], in_=w1b_d[:, :])
            g1b = sg.tile([128, H], F32, tag="g1b")
            nc.sync.dma_start(out=g1b[:, :], in_=_bcast(ln1g_d[:], 128))
            b1b = sg.tile([128, H], F32, tag="b1b")
            nc.sync.dma_start(out=b1b[:, :], in_=ln1b_d[:, :])
            w2f_t = sg.tile([128, 6, H], F32R, tag="w2f")
            nc.sync.dma_start(out=w2f_t[:, 0:3, :], in_=w2f_d[:, 0:3, :])
            nc.sync.dma_start(out=w2f_t[:, 3:6, :], in_=w2f_d[:, 3:6, :])
            b2row_t = sg.tile([1, H], F32R, tag="b2row")
            nc.sync.dma_start(out=b2row_t[:, :], in_=b2row_d[:, :])
            g2c = sg.tile([128, 2], F32, tag="g2c")
            nc.sync.dma_start(out=g2c[:, :], in_=g2c_d[:, :])
            b2c = sg.tile([128, 2], F32, tag="b2c")
            nc.sync.dma_start(out=b2c[:, :], in_=b2c_d[:, :])
            blk = sg.tile([RCH, 8, 32], F32R, tag="blk")
            nc.sync.dma_start(out=blk[:, :, :], in_=blk_d[:, :, :])

            ident = sg.tile([128, 128], F32, tag="ident")
            make_identity(nc, ident[:, :])
            eps_t = sg.tile([128, 1], F32, tag="eps")
            nc.vector.memset(eps_t[:, :], EPS)
            ones_f = sg.tile([1, 128], F32, tag="ones_f")
            nc.vector.memset(ones_f[:, :], 1.0)
            ones1 = sg.tile([1, 128], F32R, tag="ones1")
            nc.vector.tensor_copy(out=ones1[:, :], in_=ones_f[:, :])
            zerot = sg.tile([128, 2], F32, tag="zerot")
            nc.vector.memset(zerot[:, :], 0.0)

            flngb = sg.tile([BC, BC], F32, tag="flngb")
            nc.sync.dma_start(out=flngb[:, :], in_=_bcast(flng_d[:], BC))
            flnbb = sg.tile([BC, BC], F32, tag="flnbb")
            nc.sync.dma_start(out=flnbb[:, :], in_=_bcast(flnb_d[:], BC))
            fb1_t = sg.tile([BC, 1], F32, tag="fb1")
            nc.sync.dma_start(out=fb1_t[:, :], in_=fb1_d[:, :])
            fw2_t = sg.tile([BC, RMAX], F32, tag="fw2")
            nc.sync.dma_start(out=fw2_t[:, :], in_=fw2_d[:, :])
            fb2_t = sg.tile([RMAX, 1], F32, tag="fb2")
            nc.sync.dma_start(out=fb2_t[:, :], in_=fb2_d[:, :])
            fw1b_t = sg.tile([128, 2, BC], F32, tag="fw1b")
            nc.sync.dma_start(out=fw1b_t[:, :, :], in_=fw1b_d[:, :, :])
            fw1a_t = []
            for r in range(R):
                t = sg.tile([P, BC], F32, tag=f"fw1a{r}", name=f"fw1a{r}")
                nc.sync.dma_start(out=t[:, :], in_=fw1a_d[r * P:(r + 1) * P, :])
                fw1a_t.append(t)

            h_cur = sg.tile([BC, H], F32, tag="h_cur")

            # per-chunk persistents
            r2s = [sg.tile([RCH, H], F32, tag=f"r2_{j}", name=f"r2_{j}")
                   for j in range(NCHUNK)]
            yTs = [sg.tile([128, 2, RCH + 2], F32R, tag=f"yT_{j}", name=f"yT_{j}")
                   for j in range(NCHUNK)]

            # ---------- TCN L1: conv1 + LN1 + gamma/beta (gpsimd) ----------
            for j in range(NCHUNK):
                ps1 = psA.tile([RCH, H], F32, tag="ps1")
                nc.tensor.matmul(ps1[:, :], t4[:, j * RCH:(j + 1) * RCH],
                                 w1b_t[:, :], start=True, stop=True)
                y = wk.tile([RCH, H], F32, tag="y")
                nc.scalar.activation(out=y[:, :], in_=ps1[:, :], func=AF.Relu)
                yh = wk.tile([RCH, H], F32, tag="yh")
                _ln_core(nc, wk, y[:, :], yh[:, :], RCH, eps_t)
                # r2 = yh*g1 + b1 (true LN1 output) on GpSimd
                nc.gpsimd.tensor_mul(out=r2s[j][:, :], in0=yh[:, :],
                                     in1=g1b[:, :])
                nc.gpsimd.tensor_add(out=r2s[j][:, :], in0=r2s[j][:, :],
                                     in1=b1b[:, :])

            # ---------- TCN L2: transpose LN1 out into padded col space ----
            for j in range(NCHUNK):
                yT = yTs[j]
                nc.vector.tensor_copy(
                    out=yT[:, :, 0:1],
                    in_=zerot[:, :].rearrange("p (a c) -> p a c", c=1))
                nc.vector.tensor_copy(
                    out=yT[:, :, RCH + 1:RCH + 2],
                    in_=zerot[:, :].rearrange("p (a c) -> p a c", c=1))
                for cc in range(2):
                    pst = psT.tile([128, 128], F32, tag="pst")
                    nc.tensor.transpose(pst[:, :],
                                        r2s[j][:, cc * 128:(cc + 1) * 128],
                                        ident[:, :])
                    # pad rows of r2 are exactly zero (masked beta), so the
                    # full-block copy leaves conv SAME-padding zeros in place
                    nc.scalar.copy(out=yT[:, cc, 1:RCH + 1], in_=pst[:, :])

            # ---------- TCN L3: conv2 + post (lagged) + mean-w ----------
            def post(j):
                t2 = wk.tile([RCH, H], F32, tag="t2")
                nc.scalar.activation(out=t2[:, :], in_=ps2s[j][:, :],
                                     func=AF.Relu)
                y2 = wk.tile([RCH, H], F32, tag="y2")
                nc.gpsimd.tensor_add(out=y2[:, :], in0=t2[:, :],
                                     in1=r2s[j][:, :])
                y2n = wk.tile([RCH, H], F32R, tag="y2n")
                _ln_core(nc, wk, y2[:, :], y2n[:, :], RCH, eps_t)
                gi, jj = divmod(j, 8)
                if jj == 0:
                    psm32s[gi] = psM.tile([32, H], F32, tag="psm",
                                          name=f"psm32_{gi}")
                nc.tensor.matmul(psm32s[gi][:, :], blk[:, jj, :],
                                 y2n[:, :], start=(jj == 0),
                                 stop=(jj == 7))
                if jj == 7:
                    nc.scalar.copy(out=h_cur[gi * 32:(gi + 1) * 32, :],
                                   in_=psm32s[gi][:, :])

            ps2s = {}
            psm32s = {}
            for j in range(NCHUNK):
                ps2 = psB.tile([RCH, H], F32, tag="ps2")
                ps2s[j] = ps2
                kk = 0
                for cc in range(2):
                    for dw in range(3):
                        nc.tensor.matmul(ps2[:, :],
                                         yTs[j][:, cc, dw:dw + RCH],
                                         w2f_t[:, cc * 3 + dw, :],
                                         start=(kk == 0), stop=False)
                        kk += 1
                nc.tensor.matmul(ps2[:, :], ones1[:, :], b2row_t[:, :],
                                 start=False, stop=True)
                if j >= 2:
                    post(j - 2)
                    del ps2s[j - 2]
            post(NCHUNK - 2)
            post(NCHUNK - 1)

            # ---------- h_current transposed: hT[c, cc, b] (LN2 g/b folded)
            hT = sg.tile([128, 2, BC], F32, tag="hT")
            for cc in range(2):
                pst = psT.tile([128, 128], F32, tag="pst")
                nc.tensor.transpose(pst[:, :BC],
                                    h_cur[:, cc * 128:(cc + 1) * 128],
                                    ident[:BC, :BC])
                nc.scalar.activation(out=hT[:, cc, :], in_=pst[:, :BC],
                                     func=AF.Identity,
                                     scale=g2c[:, cc:cc + 1],
                                     bias=b2c[:, cc:cc + 1])

            pred_t = [sg.tile([P, BC], F32, tag=f"pred{r}", name=f"pred{r}")
                      for r in range(R)]

            # ---------- rounds ----------
            hp2 = [hT[:, 0, :], hT[:, 1, :]]
            for r in range(R):
                wih = wp.tile([128, 2, H3], F32, tag="wih")
                for q in range(4):
                    s0, s1 = q * (H3 // 2), (q + 1) * (H3 // 2)
                    cc, lo, hi = (0, s0, s1) if q < 2 else (1, s0 - H3, s1 - H3)
                    nc.sync.dma_start(out=wih[:, cc, lo:hi],
                                      in_=wih_d[r, :, cc, lo:hi])
                whh = wp.tile([128, 2, H3], F32, tag="whh")
                for q in range(4):
                    s0, s1 = q * (H3 // 2), (q + 1) * (H3 // 2)
                    cc, lo, hi = (0, s0, s1) if q < 2 else (1, s0 - H3, s1 - H3)
                    nc.sync.dma_start(out=whh[:, cc, lo:hi],
                                      in_=whh_d[r, :, cc, lo:hi])
                msgw = wp.tile([128, 2, H], F32, tag="msgw")
                nc.sync.dma_start(out=msgw[:, 0, :], in_=msgw_d[r, :, 0, :])
                nc.sync.dma_start(out=msgw[:, 1, :], in_=msgw_d[r, :, 1, :])
                hw1 = wp.tile([128, 2, H2], F32, tag="hw1")
                nc.sync.dma_start(out=hw1[:, :, :], in_=hw1_d[r, :, :, :])
                hw2 = wp.tile([H2, P], F32, tag="hw2")
                nc.sync.dma_start(out=hw2[:, :], in_=hw2_d[r, :, :])
                ball = wp.tile([128, 12], F32, tag="ball")
                nc.sync.dma_start(out=ball[:, :], in_=ball_d[r, :, :])
                brz = ball[:, 0:4]
                binb = ball[:, 4:6]
                bhnb = ball[:, 6:8]
                msgb = ball[:, 8:10]
                hb1 = ball[:, 10:11]
                hb2 = ball[:7, 11:12]

                # messages = msg_w @ h_current + msg_b  (attention sum == 1)
                vs = []
                for m in range(2):
                    pv = psA.tile([128, BC], F32, tag="ps1")
                    for cc in range(2):
                        nc.tensor.matmul(pv[:, :],
                                         msgw[:, cc, m * 128:(m + 1) * 128],
                                         hT[:, cc, :],
                                         start=(cc == 0), stop=(cc == 1))
                    v = rw.tile([128, BC], F32, tag=f"v{m}")
                    nc.scalar.activation(out=v[:, :], in_=pv[:, :],
                                         func=AF.Identity,
                                         bias=msgb[:, m:m + 1])
                    vs.append(v)

                # r/z gates: sigmoid(Whh@h + Wih@v + bih + bhh)
                gates = []
                for m in range(4):
                    pg = psA.tile([128, BC], F32, tag="ps1")
                    kk = 0
                    for wt, xs in ((whh, hp2), (wih, vs)):
                        for cc in range(2):
                            nc.tensor.matmul(pg[:, :],
                                             wt[:, cc, m * 128:(m + 1) * 128],
                                             xs[cc],
                                             start=(kk == 0), stop=(kk == 3))
                            kk += 1
                    g = rw.tile([128, BC], F32, tag=f"g{m}")
                    nc.scalar.activation(out=g[:, :], in_=pg[:, :],
                                         func=AF.Sigmoid, bias=brz[:, m:m + 1])
                    gates.append(g)

                # n gate: tanh(Wih_n@v + bin + rg * (Whh_n@h + bhn))
                ns = []
                for mi in range(2):
                    m = 4 + mi
                    ph = psT.tile([128, 128], F32, tag="pst")
                    for cc in range(2):
                        nc.tensor.matmul(ph[:, :BC],
                                         whh[:, cc, m * 128:(m + 1) * 128],
                                         hp2[cc],
                                         start=(cc == 0), stop=(cc == 1))
                    pi = psA.tile([128, BC], F32, tag="ps1")
                    for cc in range(2):
                        nc.tensor.matmul(pi[:, :],
                                         wih[:, cc, m * 128:(m + 1) * 128],
                                         vs[cc][:, :],
                                         start=(cc == 0), stop=(cc == 1))
                    hn = rw.tile([128, BC], F32, tag=f"hn{mi}")
                    nc.scalar.activation(out=hn[:, :], in_=ph[:, :BC],
                                         func=AF.Identity,
                                         bias=bhnb[:, mi:mi + 1])
                    nc.vector.tensor_mul(out=hn[:, :], in0=gates[mi][:, :],
                                         in1=hn[:, :])
                    nc.vector.tensor_add(out=hn[:, :], in0=hn[:, :],
                                         in1=pi[:, :])
                    n_t = rw.tile([128, BC], F32, tag=f"n{mi}")
                    nc.scalar.activation(out=n_t[:, :], in_=hn[:, :],
                                         func=AF.Tanh, bias=binb[:, mi:mi + 1])
                    ns.append(n_t)

                # h' = n + z*(h - n)
                hnew = rw.tile([128, 2, BC], F32, tag="hstate")
                for mi in range(2):
                    d = rw.tile([128, BC], F32, tag=f"d{mi}")
                    nc.vector.tensor_sub(out=d[:, :], in0=hp2[mi],
                                         in1=ns[mi][:, :])
                    nc.vector.tensor_mul(out=d[:, :], in0=gates[2 + mi][:, :],
                                         in1=d[:, :])
                    nc.vector.tensor_add(out=hnew[:, mi, :], in0=ns[mi][:, :],
                                         in1=d[:, :])
                hp2 = [hnew[:, 0, :], hnew[:, 1, :]]

                # head: pred = head_w2 @ relu(head_w1 @ h' + b1) + b2
                pp = psB.tile([RCH, H], F32, tag="ps2")
                for cc in range(2):
                    nc.tensor.matmul(pp[:H2, :BC], hw1[:, cc, :], hp2[cc],
                                     start=(cc == 0), stop=(cc == 1))
                p1 = rw.tile([H2, BC], F32, tag="p1")
                nc.scalar.activation(out=p1[:, :], in_=pp[:H2, :BC],
                                     func=AF.Relu, bias=hb1[:H2, 0:1])
                pq = psB.tile([RCH, H], F32, tag="ps2")
                nc.tensor.matmul(pq[:P, :BC], hw2[:, :], p1[:, :],
                                 start=True, stop=True)
                nc.scalar.activation(out=pred_t[r][:, :], in_=pq[:P, :BC],
                                     func=AF.Identity, bias=hb2[:, 0:1])

            # ---------- fusion ----------
            phf = psA.tile([128, BC], F32, tag="ps1")
            nk = R + 2
            kk = 0
            for r in range(R):
                nc.tensor.matmul(phf[:BC, :], fw1a_t[r][:, :],
                                 pred_t[r][:, :],
                                 start=(kk == 0), stop=False)
                kk += 1
            for cc in range(2):
                nc.tensor.matmul(phf[:BC, :], fw1b_t[:, cc, :], hp2[cc],
                                 start=(kk == 0), stop=(kk == nk - 1))
                kk += 1
            hfT = rw.tile([BC, BC], F32, tag="hfT")
            nc.scalar.activation(out=hfT[:, :], in_=phf[:BC, :],
                                 func=AF.Relu, bias=fb1_t[:, 0:1])
            pstf = psT.tile([128, 128], F32, tag="pst")
            nc.tensor.transpose(pstf[:BC, :BC], hfT[:, :], ident[:BC, :BC])
            hfr = rw.tile([BC, BC], F32, tag="hfr")
            nc.scalar.copy(out=hfr[:, :], in_=pstf[:BC, :BC])
            hfn0 = rw.tile([BC, BC], F32, tag="hfn0")
            _ln_core(nc, wk, hfr[:, :], hfn0[:, :], BC, eps_t)
            hfn = rw.tile([BC, BC], F32, tag="hfn")
            nc.vector.tensor_mul(out=hfn0[:, :], in0=hfn0[:, :],
                                 in1=flngb[:, :])
            nc.vector.tensor_add(out=hfn[:, :], in0=hfn0[:, :],
                                 in1=flnbb[:, :])
            pstg = psT.tile([128, 128], F32, tag="pst")
            nc.tensor.transpose(pstg[:BC, :BC], hfn[:, :], ident[:BC, :BC])
            hfnT = rw.tile([BC, BC], F32, tag="hfnT")
            nc.scalar.copy(out=hfnT[:, :], in_=pstg[:BC, :BC])

            plg = psA.tile([128, BC], F32, tag="ps1")
            nc.tensor.matmul(plg[:RMAX, :], fw2_t[:, :], hfnT[:, :],
                             start=True, stop=True)
            lgT = rw.tile([RMAX, BC], F32, tag="lgT")
            nc.scalar.activation(out=lgT[:, :], in_=plg[:RMAX, :],
                                 func=AF.Identity, bias=fb2_t[:, 0:1])
            psth = psT.tile([128, 128], F32, tag="pst")
            nc.tensor.transpose(psth[:BC, :RMAX], lgT[:, :],
                                ident[:RMAX, :RMAX])
            lg = rw.tile([BC, RMAX], F32, tag="lg")
            nc.scalar.copy(out=lg[:, :], in_=psth[:BC, :RMAX])

            mx = rw.tile([BC, 1], F32, tag="mx")
            nc.vector.reduce_max(out=mx[:, :], in_=lg[:, :], axis=AX.X)
            nc.vector.tensor_scalar(out=lg[:, :], in0=lg[:, :],
                                    scalar1=mx[:, 0:1], scalar2=None,
                                    op0=OP.subtract)
            ex = rw.tile([BC, RMAX], F32, tag="ex")
            sm = rw.tile([BC, 1], F32, tag="sm")
            nc.scalar.activation(out=ex[:, :], in_=lg[:, :], func=AF.Exp,
                                 accum_out=sm[:, 0:1])
            nc.vector.reciprocal(out=sm[:, :], in_=sm[:, :])
            fwr = rw.tile([BC, RMAX], F32, tag="fwr")
            nc.vector.tensor_scalar_mul(out=fwr[:, :], in0=ex[:, :],
                                        scalar1=sm[:, 0:1])
            nc.sync.dma_start(out=fw_d[:, :], in_=fwr[:, :])

            prs = rw.tile([BC, R * P], F32, tag="prs")
            for r in range(R):
                pstp = psT.tile([128, 128], F32, tag="pst")
                nc.tensor.transpose(pstp[:BC, :P], pred_t[r][:, :],
                                    ident[:P, :P])
                nc.scalar.copy(out=prs[:, r * P:(r + 1) * P],
                               in_=pstp[:BC, :P])
            nc.sync.dma_start(out=pers_d[:, :], in_=prs[:, :])

            fin = rw.tile([BC, P], F32, tag="fin")
            ftmp = rw.tile([BC, P], F32, tag="ftmp")
            for r in range(R):
                dst = fin if r == 0 else ftmp
                nc.vector.tensor_scalar_mul(out=dst[:, :],
                                            in0=prs[:, r * P:(r + 1) * P],
                                            scalar1=fwr[:, r:r + 1])
                if r > 0:
                    nc.vector.tensor_add(out=fin[:, :], in0=fin[:, :],
                                         in1=ftmp[:, :])
            nc.sync.dma_start(out=fin_d[:, :], in_=fin[:, :])

    nc.compile()
    return nc


_COMPILED = {}


def _get_compiled(R):
    if R not in _COMPILED:
        _COMPILED[R] = _build(R)
    return _COMPILED[R]


_BLK = np.zeros((RCH, 8, 32), np.float32)
for _p in range(RCH):
    if _p % WP < W:
        for _j in range(8):
            _BLK[_p, _j, 4 * _j + _p // WP] = 1.0


def _prep_x4(xs):
    x4 = np.zeros((4, BC, WP), np.float32)
    x4[0, :, 1:W] = xs[:, :W - 1]
    x4[1, :, :W] = xs
    x4[2, :, :W - 1] = xs[:, 1:]
    x4[3, :, :W] = 1.0
    return x4


def _prep_weights(inp, R):
    f = lambda a: np.ascontiguousarray(np.asarray(a, dtype=np.float32))
    conv1_w = f(inp["conv1_w"]); conv1_b = f(inp["conv1_b"])
    conv2_w = f(inp["conv2_w"]); conv2_b = f(inp["conv2_b"])
    w1b = np.zeros((4, H), np.float32)
    w1b[0:3] = conv1_w[:, 0, :].T
    w1b[3] = conv1_b
    w2f = conv2_w.transpose(1, 2, 0).reshape(2, 128, 3, H) \
        .transpose(1, 0, 2, 3).reshape(128, 6, H)
    gw = lambda a: f(a)[:R].transpose(0, 2, 1).reshape(R, 2, 128, -1) \
        .transpose(0, 2, 1, 3)
    bih = f(inp["gru_bih"])[:R]; bhh = f(inp["gru_bhh"])[:R]
    ch = lambda a: a.reshape(R, 2, 128).transpose(0, 2, 1)
    ball = np.zeros((R, 128, 12), np.float32)
    ball[:, :, 0:4] = (bih + bhh)[:, :2 * H].reshape(R, 4, 128) \
        .transpose(0, 2, 1)
    ball[:, :, 4:6] = ch(bih[:, 2 * H:])
    ball[:, :, 6:8] = ch(bhh[:, 2 * H:])
    ball[:, :, 8:10] = ch(f(inp["msg_b"])[:R])
    ball[:, :, 10] = f(inp["head_b1"])[:R]
    ball[:, :7, 11] = f(inp["head_b2"])[:R]
    ln2_g = f(inp["ln2_g"]); ln2_b = f(inp["ln2_b"])
    fusion_w1 = f(inp["fusion_w1"])
    m = {
        "w1b": w1b,
        "ln1g": f(inp["ln1_g"]),
        "b1bm": np.ascontiguousarray(
            np.where((np.arange(128) % WP < W)[:, None],
                     f(inp["ln1_b"])[None, :], 0.0).astype(np.float32)),
        "g2c": np.ascontiguousarray((ln2_g / W).reshape(2, 128).T),
        "b2c": np.ascontiguousarray(ln2_b.reshape(2, 128).T),
        "w2f": np.ascontiguousarray(w2f),
        "b2row": conv2_b[None, :],
        "wihT": np.ascontiguousarray(gw(inp["gru_wih"])),
        "whhT": np.ascontiguousarray(gw(inp["gru_whh"])),
        "msgwT": np.ascontiguousarray(gw(inp["msg_w"])),
        "hw1T": np.ascontiguousarray(gw(inp["head_w1"])),
        "hw2T": np.ascontiguousarray(f(inp["head_w2"])[:R].transpose(0, 2, 1)),
        "ball": ball,
        "fw1aT": np.ascontiguousarray(fusion_w1[:, :R * P].T),
        "fw1bT": np.ascontiguousarray(
            fusion_w1[:, RMAX * P:].T.reshape(2, 128, BC).transpose(1, 0, 2)),
        "fb1": f(inp["fusion_b1"])[:, None],
        "flng": f(inp["fusion_ln_g"]), "flnb": f(inp["fusion_ln_b"]),
        "fw2T": np.ascontiguousarray(f(inp["fusion_w2"]).T),
        "fb2": f(inp["fusion_b2"])[:, None],
        "blk": _BLK,
    }
    return m


def run_on_device(inputs, trace=False):
    """Shard, run the bass kernel on 8 cores, gather. Returns (outs, bkr)."""
    R = int(np.asarray(inputs["R"]))
    nc = _get_compiled(R)
    shared = _prep_weights(inputs, R)
    x = np.ascontiguousarray(np.asarray(inputs["x"], dtype=np.float32))
    in_maps = []
    for i in range(NCORES):
        mm = dict(shared)
        mm["x4"] = _prep_x4(x[i * BC:(i + 1) * BC])
        in_maps.append(mm)
    bkr = run_bass_kernel_spmd(nc, in_maps, core_ids=list(range(NCORES)),
                               trace=trace)
    res = bkr.results
    final = np.concatenate([res[i]["final"] for i in range(NCORES)], axis=0)
    pa = np.concatenate([res[i]["pers_act"] for i in range(NCORES)], axis=0)
    fw = np.concatenate([res[i]["fw"] for i in range(NCORES)], axis=0)
    pers = np.zeros((B, RMAX, P), np.float32)
    pers[:, :R, :] = pa.reshape(B, R, P)
    return (final.astype(np.float32), pers, fw.astype(np.float32)), bkr


def kernel(**inputs):
    outs, _ = run_on_device(inputs, trace=False)
    return outs


# revision 13
# speedup vs baseline: 2.8211x; 1.1583x over previous
"""Trainium2 Bass kernel for nn_DHCSTGCN (TCN encoder + GRU rounds + fusion).

Math note: in the reference, the confidence-modulation / attention block only
reaches the output through att.sum(-1), which is the sum of a softmax == 1
(up to fp32 rounding ~1e-6).  `messages` therefore equals the msg-linear
output `v` exactly, and the whole [B,N,H/2] block (and g/m/c/r_vec inputs)
drops out of the computation.

Sharding: data-parallel over batch, 64 rows per core on 8 cores; all weights
replicated.  Outputs are gathered/concatenated on host.

Layout: the TCN stage runs in a padded row space — each batch occupies 32
rows (30 time steps + 2 zero rows), so a 4-batch chunk is exactly 128 rows.
The zero columns double as conv SAME-padding when the transposed activations
are read with a +-1 shifted stride-1 slice, keeping every matmul stationary
operand a single free dimension.

Perf notes: big matmuls (moving dim 256) run as float32r (1 cycle/row vs 4
for fp32); the TCN is emitted phase-major so each engine's stream is dense;
LayerNorm gamma/beta of LN2 are folded into the h_current transpose-copy
(per-partition scale/bias on the ACT engine); LN1 gamma/beta and the
residual add run on the otherwise-idle GpSimd engine.
"""

import numpy as np

import concourse.bacc as bacc
import concourse.bass as bass
import concourse.tile as tile
from concourse import mybir
from concourse.masks import make_identity
from concourse.bass_utils import run_bass_kernel_spmd

F32 = mybir.dt.float32
F32R = mybir.dt.float32r
AF = mybir.ActivationFunctionType
OP = mybir.AluOpType
AX = mybir.AxisListType

B, N, H, W, P, RMAX = 512, 512, 256, 30, 7, 30
NCORES = 8
BC = B // NCORES      # 64 batch rows per core
WP = 32               # padded time steps per batch
CB = 4                # batches per row-chunk
RCH = CB * WP         # 128 padded rows per chunk
NCHUNK = BC // CB     # 16 chunks
H2 = H // 2           # 128
H3 = 3 * H            # 768
EPS = 1e-5


def _bcast(ap, parts):
    """Partition-broadcast a DRAM AP: [d...] -> [parts, d...] with step 0."""
    return bass.AP(tensor=ap.tensor, offset=ap.offset,
                   ap=[[0, parts]] + [list(d) for d in ap.ap])


def _r(ap):
    return ap.bitcast(F32R)


def _ln_core(nc, wk, src, dst, rows, eps_t):
    """y_hat = (src - mean)/sqrt(var+eps) over free dim (src clobbered)."""
    st = wk.tile([128, 6], F32, tag="lnst")
    nc.vector.bn_stats(out=st[:rows], in_=src)
    mv = wk.tile([128, 2], F32, tag="lnmv")
    nc.vector.bn_aggr(out=mv[:rows], in_=st[:rows])
    sd = wk.tile([128, 1], F32, tag="lnsd")
    nc.scalar.activation(out=sd[:rows], in_=mv[:rows, 1:2], func=AF.Sqrt,
                         bias=eps_t[:rows], scale=1.0)
    nc.vector.reciprocal(out=sd[:rows], in_=sd[:rows])
    nc.vector.tensor_scalar(out=dst, in0=src, scalar1=mv[:rows, 0:1],
                            scalar2=sd[:rows], op0=OP.subtract, op1=OP.mult)


def _build(R):
    nc = bacc.Bacc("TRN2", target_bir_lowering=False, debug=False)

    di = lambda name, shape: nc.dram_tensor(name, shape, F32, kind="ExternalInput")
    dir_ = lambda name, shape: nc.dram_tensor(name, shape, F32R, kind="ExternalInput")
    do = lambda name, shape: nc.dram_tensor(name, shape, F32, kind="ExternalOutput")

    x4_d = dir_("x4", [4, BC, WP])
    blk_d = dir_("blk", [RCH, 8, 32])
    w1b_d = dir_("w1b", [4, H])
    ln1g_d = di("ln1g", [H]); ln1b_d = di("b1bm", [128, H])
    g2c_d = di("g2c", [128, 2]); b2c_d = di("b2c", [128, 2])
    w2f_d = dir_("w2f", [128, 6, H])
    b2row_d = dir_("b2row", [1, H])
    wihm_d = di("wihmT", [R, 128, 2, H3])
    whh_d = di("whhT", [R, 128, 2, H3])
    w0rz_d = di("w0rzT", [128, 2, 2 * H])
    hw1_d = di("hw1T", [R, 128, 2, H2])
    hw2_d = di("hw2T", [R, H2, P])
    ball_d = di("ball", [R, 128, 12])
    fw1a_d = di("fw1aT", [R * P, BC])
    fw1b_d = di("fw1bT", [128, 2, BC])
    fb1_d = di("fb1", [BC, 1])
    flng_d = di("flng", [BC]); flnb_d = di("flnb", [BC])
    fw2_d = di("fw2T", [BC, RMAX])
    fb2_d = di("fb2", [RMAX, 1])

    fin_d = do("final", [BC, P])
    pers_d = do("pers_act", [BC, R * P])
    fw_d = do("fw", [BC, RMAX])

    with tile.TileContext(nc) as tc:
        with (
            tc.tile_pool(name="singles", bufs=1) as sg,
            tc.tile_pool(name="wk", bufs=6) as wk,
            tc.tile_pool(name="wp", bufs=max(2, R)) as wp,
            tc.tile_pool(name="rw", bufs=3) as rw,
            tc.tile_pool(name="psA", bufs=2, space="PSUM") as psA,
            tc.tile_pool(name="psB", bufs=3, space="PSUM") as psB,
            tc.tile_pool(name="psT", bufs=2, space="PSUM") as psT,
            tc.tile_pool(name="psM", bufs=1, space="PSUM") as psM,
        ):
            # ---------- critical-path inputs first ----------
            t4 = sg.tile([4, BC * WP], F32R, tag="t4")
            nc.sync.dma_start(out=t4[:, :],
                              in_=x4_d[:, :, :].rearrange("k b w -> k (b w)"))
            w1b_t = sg.tile([4, H], F32R, tag="w1b")
            nc.sync.dma_start(out=w1b_t[:, :], in_=w1b_d[:, :])
            g1b = sg.tile([128, H], F32, tag="g1b")
            nc.sync.dma_start(out=g1b[:, :], in_=_bcast(ln1g_d[:], 128))
            b1b = sg.tile([128, H], F32, tag="b1b")
            nc.sync.dma_start(out=b1b[:, :], in_=ln1b_d[:, :])
            w2f_t = sg.tile([128, 6, H], F32R, tag="w2f")
            nc.sync.dma_start(out=w2f_t[:, 0:3, :], in_=w2f_d[:, 0:3, :])
            nc.sync.dma_start(out=w2f_t[:, 3:6, :], in_=w2f_d[:, 3:6, :])
            b2row_t = sg.tile([1, H], F32R, tag="b2row")
            nc.sync.dma_start(out=b2row_t[:, :], in_=b2row_d[:, :])
            g2c = sg.tile([128, 2], F32, tag="g2c")
            nc.sync.dma_start(out=g2c[:, :], in_=g2c_d[:, :])
            b2c = sg.tile([128, 2], F32, tag="b2c")
            nc.sync.dma_start(out=b2c[:, :], in_=b2c_d[:, :])
            blk = sg.tile([RCH, 8, 32], F32R, tag="blk")
            nc.sync.dma_start(out=blk[:, :, :], in_=blk_d[:, :, :])

            ident = sg.tile([128, 128], F32, tag="ident")
            make_identity(nc, ident[:, :])
            eps_t = sg.tile([128, 1], F32, tag="eps")
            nc.vector.memset(eps_t[:, :], EPS)
            ones_f = sg.tile([1, 128], F32, tag="ones_f")
            nc.vector.memset(ones_f[:, :], 1.0)
            ones1 = sg.tile([1, 128], F32R, tag="ones1")
            nc.vector.tensor_copy(out=ones1[:, :], in_=ones_f[:, :])
            zerot = sg.tile([128, 2], F32, tag="zerot")
            nc.vector.memset(zerot[:, :], 0.0)

            flngb = sg.tile([BC, BC], F32, tag="flngb")
            nc.sync.dma_start(out=flngb[:, :], in_=_bcast(flng_d[:], BC))
            flnbb = sg.tile([BC, BC], F32, tag="flnbb")
            nc.sync.dma_start(out=flnbb[:, :], in_=_bcast(flnb_d[:], BC))
            fb1_t = sg.tile([BC, 1], F32, tag="fb1")
            nc.sync.dma_start(out=fb1_t[:, :], in_=fb1_d[:, :])
            fw2_t = sg.tile([BC, RMAX], F32, tag="fw2")
            nc.sync.dma_start(out=fw2_t[:, :], in_=fw2_d[:, :])
            fb2_t = sg.tile([RMAX, 1], F32, tag="fb2")
            nc.sync.dma_start(out=fb2_t[:, :], in_=fb2_d[:, :])
            fw1b_t = sg.tile([128, 2, BC], F32, tag="fw1b")
            nc.sync.dma_start(out=fw1b_t[:, :, :], in_=fw1b_d[:, :, :])
            fw1a_t = []
            for r in range(R):
                t = sg.tile([P, BC], F32, tag=f"fw1a{r}", name=f"fw1a{r}")
                nc.sync.dma_start(out=t[:, :], in_=fw1a_d[r * P:(r + 1) * P, :])
                fw1a_t.append(t)

            h_cur = sg.tile([BC, H], F32, tag="h_cur")

            # per-chunk persistents
            r2s = [sg.tile([RCH, H], F32, tag=f"r2_{j}", name=f"r2_{j}")
                   for j in range(NCHUNK)]
            yTs = [sg.tile([128, 2, RCH + 2], F32R, tag=f"yT_{j}", name=f"yT_{j}")
                   for j in range(NCHUNK)]

            # ---------- TCN L1: conv1 + LN1 + gamma/beta (gpsimd) ----------
            for j in range(NCHUNK):
                ps1 = psA.tile([RCH, H], F32, tag="ps1")
                nc.tensor.matmul(ps1[:, :], t4[:, j * RCH:(j + 1) * RCH],
                                 w1b_t[:, :], start=True, stop=True)
                y = wk.tile([RCH, H], F32, tag="y")
                nc.scalar.activation(out=y[:, :], in_=ps1[:, :], func=AF.Relu)
                yh = wk.tile([RCH, H], F32, tag="yh")
                _ln_core(nc, wk, y[:, :], yh[:, :], RCH, eps_t)
                # r2 = yh*g1 + b1 (true LN1 output) on GpSimd
                nc.gpsimd.tensor_mul(out=r2s[j][:, :], in0=yh[:, :],
                                     in1=g1b[:, :])
                nc.vector.tensor_add(out=r2s[j][:, :], in0=r2s[j][:, :],
                                     in1=b1b[:, :])

            # ---------- TCN L2: transpose LN1 out into padded col space ----
            for j in range(NCHUNK):
                yT = yTs[j]
                nc.vector.tensor_copy(
                    out=yT[:, :, 0:1],
                    in_=zerot[:, :].rearrange("p (a c) -> p a c", c=1))
                nc.vector.tensor_copy(
                    out=yT[:, :, RCH + 1:RCH + 2],
                    in_=zerot[:, :].rearrange("p (a c) -> p a c", c=1))
                for cc in range(2):
                    pst = psT.tile([128, 128], F32, tag="pst")
                    nc.tensor.transpose(pst[:, :],
                                        r2s[j][:, cc * 128:(cc + 1) * 128],
                                        ident[:, :])
                    # pad rows of r2 are exactly zero (masked beta), so the
                    # full-block copy leaves conv SAME-padding zeros in place
                    nc.scalar.copy(out=yT[:, cc, 1:RCH + 1], in_=pst[:, :])

            # ---------- TCN L3: conv2 + post (lagged) + mean-w ----------
            def post(j):
                t2 = wk.tile([RCH, H], F32, tag="t2")
                nc.scalar.activation(out=t2[:, :], in_=ps2s[j][:, :],
                                     func=AF.Relu)
                y2 = wk.tile([RCH, H], F32, tag="y2")
                nc.gpsimd.tensor_add(out=y2[:, :], in0=t2[:, :],
                                     in1=r2s[j][:, :])
                y2n = wk.tile([RCH, H], F32R, tag="y2n")
                _ln_core(nc, wk, y2[:, :], y2n[:, :], RCH, eps_t)
                gi, jj = divmod(j, 8)
                if jj == 0:
                    psm32s[gi] = psM.tile([32, H], F32, tag="psm",
                                          name=f"psm32_{gi}")
                nc.tensor.matmul(psm32s[gi][:, :], blk[:, jj, :],
                                 y2n[:, :], start=(jj == 0),
                                 stop=(jj == 7))
                if jj == 7:
                    nc.scalar.copy(out=h_cur[gi * 32:(gi + 1) * 32, :],
                                   in_=psm32s[gi][:, :])

            ps2s = {}
            psm32s = {}
            for j in range(NCHUNK):
                ps2 = psB.tile([RCH, H], F32, tag="ps2")
                ps2s[j] = ps2
                kk = 0
                for cc in range(2):
                    for dw in range(3):
                        nc.tensor.matmul(ps2[:, :],
                                         yTs[j][:, cc, dw:dw + RCH],
                                         w2f_t[:, cc * 3 + dw, :],
                                         start=(kk == 0), stop=False)
                        kk += 1
                nc.tensor.matmul(ps2[:, :], ones1[:, :], b2row_t[:, :],
                                 start=False, stop=True)
                if j >= 2:
                    post(j - 2)
                    del ps2s[j - 2]
            post(NCHUNK - 2)
            post(NCHUNK - 1)

            # ---------- h_current transposed: hT[c, cc, b] (LN2 g/b folded)
            hT = sg.tile([128, 2, BC], F32, tag="hT")
            for cc in range(2):
                pst = psT.tile([128, 128], F32, tag="pst")
                nc.tensor.transpose(pst[:, :BC],
                                    h_cur[:, cc * 128:(cc + 1) * 128],
                                    ident[:BC, :BC])
                nc.scalar.activation(out=hT[:, cc, :], in_=pst[:, :BC],
                                     func=AF.Identity,
                                     scale=g2c[:, cc:cc + 1],
                                     bias=b2c[:, cc:cc + 1])

            pred_t = [sg.tile([P, BC], F32, tag=f"pred{r}", name=f"pred{r}")
                      for r in range(R)]

            # ---------- rounds ----------
            # gi_r = Wih_r@(Mw_r@h_cur + mb_r) + bih_r is host-folded to
            # wihm_r@h_cur + bia_r; round 0 merges wihm_0+whh_0 for r/z
            # (h_state0 == h_current); later rounds hoist their gi parts.
            hp2 = [hT[:, 0, :], hT[:, 1, :]]

            wihms, whhs, hw1s, hw2s, balls = [], [], [], [], []
            for r in range(R):
                wihm = wp.tile([128, 2, H3], F32, tag="wihm")
                for q in range(4):
                    s0, s1 = q * (H3 // 2), (q + 1) * (H3 // 2)
                    cc, lo, hi = (0, s0, s1) if q < 2 else (1, s0 - H3, s1 - H3)
                    nc.sync.dma_start(out=wihm[:, cc, lo:hi],
                                      in_=wihm_d[r, :, cc, lo:hi])
                whh = wp.tile([128, 2, H3], F32, tag="whh")
                for q in range(4):
                    s0, s1 = q * (H3 // 2), (q + 1) * (H3 // 2)
                    cc, lo, hi = (0, s0, s1) if q < 2 else (1, s0 - H3, s1 - H3)
                    nc.sync.dma_start(out=whh[:, cc, lo:hi],
                                      in_=whh_d[r, :, cc, lo:hi])
                hw1 = wp.tile([128, 2, H2], F32, tag="hw1")
                nc.sync.dma_start(out=hw1[:, :, :], in_=hw1_d[r, :, :, :])
                hw2 = wp.tile([H2, P], F32, tag="hw2")
                nc.sync.dma_start(out=hw2[:, :], in_=hw2_d[r, :, :])
                ball = wp.tile([128, 12], F32, tag="ball")
                nc.sync.dma_start(out=ball[:, :], in_=ball_d[r, :, :])
                wihms.append(wihm); whhs.append(whh)
                hw1s.append(hw1); hw2s.append(hw2); balls.append(ball)
            w0rz = wp.tile([128, 2, 2 * H], F32, tag="w0rz")
            nc.sync.dma_start(out=w0rz[:, :, :], in_=w0rz_d[:, :, :])

            # hoisted gi parts for rounds >= 1 (depend only on h_current)
            gia = {}
            for r in range(1, R):
                for m in range(6):
                    pgi = psB.tile([RCH, H], F32, tag="ps2")
                    for cc in range(2):
                        nc.tensor.matmul(pgi[:, :BC],
                                         wihms[r][:, cc,
                                                  m * 128:(m + 1) * 128],
                                         hT[:, cc, :],
                                         start=(cc == 0), stop=(cc == 1))
                    bcol = m if m < 4 else 4 + (m - 4)
                    gt = rw.tile([128, BC], F32, tag=f"gia{m}")
                    nc.scalar.activation(out=gt[:, :], in_=pgi[:, :BC],
                                         func=AF.Identity,
                                         bias=balls[r][:, bcol:bcol + 1])
                    gia[(r, m)] = gt

            for r in range(R):
                ball = balls[r]
                brz = ball[:, 0:4]
                binb = ball[:, 4:6]
                bhnb = ball[:, 6:8]
                hb1 = ball[:, 10:11]
                hb2 = ball[:7, 11:12]

                # r/z gates
                gates = []
                for m in range(4):
                    pg = psA.tile([128, BC], F32, tag="ps1")
                    if r == 0:
                        for cc in range(2):
                            nc.tensor.matmul(pg[:, :],
                                             w0rz[:, cc,
                                                  m * 128:(m + 1) * 128],
                                             hp2[cc],
                                             start=(cc == 0), stop=(cc == 1))
                        g = rw.tile([128, BC], F32, tag=f"g{m}")
                        nc.scalar.activation(out=g[:, :], in_=pg[:, :],
                                             func=AF.Sigmoid,
                                             bias=brz[:, m:m + 1])
                    else:
                        for cc in range(2):
                            nc.tensor.matmul(pg[:, :],
                                             whhs[r][:, cc,
                                                     m * 128:(m + 1) * 128],
                                             hp2[cc],
                                             start=(cc == 0), stop=(cc == 1))
                        gp = rw.tile([128, BC], F32, tag=f"gp{m}")
                        nc.vector.tensor_add(out=gp[:, :], in0=pg[:, :],
                                             in1=gia[(r, m)][:, :])
                        g = rw.tile([128, BC], F32, tag=f"g{m}")
                        nc.scalar.activation(out=g[:, :], in_=gp[:, :],
                                             func=AF.Sigmoid)
                    gates.append(g)

                # n gate: tanh(gi_n + rg * (Whh_n@h + bhn))
                ns = []
                for mi in range(2):
                    m = 4 + mi
                    ph = psT.tile([128, 128], F32, tag="pst")
                    for cc in range(2):
                        nc.tensor.matmul(ph[:, :BC],
                                         whhs[r][:, cc,
                                                 m * 128:(m + 1) * 128],
                                         hp2[cc],
                                         start=(cc == 0), stop=(cc == 1))
                    hn = rw.tile([128, BC], F32, tag=f"hn{mi}")
                    nc.scalar.activation(out=hn[:, :], in_=ph[:, :BC],
                                         func=AF.Identity,
                                         bias=bhnb[:, mi:mi + 1])
                    nc.vector.tensor_mul(out=hn[:, :], in0=gates[mi][:, :],
                                         in1=hn[:, :])
                    if r == 0:
                        pi = psA.tile([128, BC], F32, tag="ps1")
                        for cc in range(2):
                            nc.tensor.matmul(pi[:, :],
                                             wihms[0][:, cc,
                                                      m * 128:(m + 1) * 128],
                                             hp2[cc],
                                             start=(cc == 0), stop=(cc == 1))
                        nc.vector.tensor_add(out=hn[:, :], in0=hn[:, :],
                                             in1=pi[:, :])
                        n_t = rw.tile([128, BC], F32, tag=f"n{mi}")
                        nc.scalar.activation(out=n_t[:, :], in_=hn[:, :],
                                             func=AF.Tanh,
                                             bias=binb[:, mi:mi + 1])
                    else:
                        nc.vector.tensor_add(out=hn[:, :], in0=hn[:, :],
                                             in1=gia[(r, m)][:, :])
                        n_t = rw.tile([128, BC], F32, tag=f"n{mi}")
                        nc.scalar.activation(out=n_t[:, :], in_=hn[:, :],
                                             func=AF.Tanh, bias=0.0)
                    ns.append(n_t)

                # h' = n + z*(h - n)
                hnew = rw.tile([128, 2, BC], F32, tag="hstate")
                for mi in range(2):
                    d = rw.tile([128, BC], F32, tag=f"d{mi}")
                    nc.vector.tensor_sub(out=d[:, :], in0=hp2[mi],
                                         in1=ns[mi][:, :])
                    nc.vector.tensor_mul(out=d[:, :], in0=gates[2 + mi][:, :],
                                         in1=d[:, :])
                    nc.vector.tensor_add(out=hnew[:, mi, :], in0=ns[mi][:, :],
                                         in1=d[:, :])
                hp2 = [hnew[:, 0, :], hnew[:, 1, :]]

                # head: pred = head_w2 @ relu(head_w1 @ h' + b1) + b2
                pp = psB.tile([RCH, H], F32, tag="ps2")
                for cc in range(2):
                    nc.tensor.matmul(pp[:H2, :BC], hw1s[r][:, cc, :], hp2[cc],
                                     start=(cc == 0), stop=(cc == 1))
                p1 = rw.tile([H2, BC], F32, tag="p1")
                nc.scalar.activation(out=p1[:, :], in_=pp[:H2, :BC],
                                     func=AF.Relu, bias=hb1[:H2, 0:1])
                pq = psB.tile([RCH, H], F32, tag="ps2")
                nc.tensor.matmul(pq[:P, :BC], hw2s[r][:, :], p1[:, :],
                                 start=True, stop=True)
                nc.scalar.activation(out=pred_t[r][:, :], in_=pq[:P, :BC],
                                     func=AF.Identity, bias=hb2[:, 0:1])

            # ---------- fusion ----------
            phf = psA.tile([128, BC], F32, tag="ps1")
            nk = R + 2
            kk = 0
            for r in range(R):
                nc.tensor.matmul(phf[:BC, :], fw1a_t[r][:, :],
                                 pred_t[r][:, :],
                                 start=(kk == 0), stop=False)
                kk += 1
            for cc in range(2):
                nc.tensor.matmul(phf[:BC, :], fw1b_t[:, cc, :], hp2[cc],
                                 start=(kk == 0), stop=(kk == nk - 1))
                kk += 1
            hfT = rw.tile([BC, BC], F32, tag="hfT")
            nc.scalar.activation(out=hfT[:, :], in_=phf[:BC, :],
                                 func=AF.Relu, bias=fb1_t[:, 0:1])
            pstf = psT.tile([128, 128], F32, tag="pst")
            nc.tensor.transpose(pstf[:BC, :BC], hfT[:, :], ident[:BC, :BC])
            hfr = rw.tile([BC, BC], F32, tag="hfr")
            nc.scalar.copy(out=hfr[:, :], in_=pstf[:BC, :BC])
            hfn0 = rw.tile([BC, BC], F32, tag="hfn0")
            _ln_core(nc, wk, hfr[:, :], hfn0[:, :], BC, eps_t)
            hfn = rw.tile([BC, BC], F32, tag="hfn")
            nc.vector.tensor_mul(out=hfn0[:, :], in0=hfn0[:, :],
                                 in1=flngb[:, :])
            nc.vector.tensor_add(out=hfn[:, :], in0=hfn0[:, :],
                                 in1=flnbb[:, :])
            pstg = psT.tile([128, 128], F32, tag="pst")
            nc.tensor.transpose(pstg[:BC, :BC], hfn[:, :], ident[:BC, :BC])
            hfnT = rw.tile([BC, BC], F32, tag="hfnT")
            nc.scalar.copy(out=hfnT[:, :], in_=pstg[:BC, :BC])

            plg = psA.tile([128, BC], F32, tag="ps1")
            nc.tensor.matmul(plg[:RMAX, :], fw2_t[:, :], hfnT[:, :],
                             start=True, stop=True)
            lgT = rw.tile([RMAX, BC], F32, tag="lgT")
            nc.scalar.activation(out=lgT[:, :], in_=plg[:RMAX, :],
                                 func=AF.Identity, bias=fb2_t[:, 0:1])
            psth = psT.tile([128, 128], F32, tag="pst")
            nc.tensor.transpose(psth[:BC, :RMAX], lgT[:, :],
                                ident[:RMAX, :RMAX])
            lg = rw.tile([BC, RMAX], F32, tag="lg")
            nc.scalar.copy(out=lg[:, :], in_=psth[:BC, :RMAX])

            mx = rw.tile([BC, 1], F32, tag="mx")
            nc.vector.reduce_max(out=mx[:, :], in_=lg[:, :], axis=AX.X)
            nc.vector.tensor_scalar(out=lg[:, :], in0=lg[:, :],
                                    scalar1=mx[:, 0:1], scalar2=None,
                                    op0=OP.subtract)
            ex = rw.tile([BC, RMAX], F32, tag="ex")
            sm = rw.tile([BC, 1], F32, tag="sm")
            nc.scalar.activation(out=ex[:, :], in_=lg[:, :], func=AF.Exp,
                                 accum_out=sm[:, 0:1])
            nc.vector.reciprocal(out=sm[:, :], in_=sm[:, :])
            fwr = rw.tile([BC, RMAX], F32, tag="fwr")
            nc.vector.tensor_scalar_mul(out=fwr[:, :], in0=ex[:, :],
                                        scalar1=sm[:, 0:1])
            nc.sync.dma_start(out=fw_d[:, :], in_=fwr[:, :])

            prs = rw.tile([BC, R * P], F32, tag="prs")
            for r in range(R):
                pstp = psT.tile([128, 128], F32, tag="pst")
                nc.tensor.transpose(pstp[:BC, :P], pred_t[r][:, :],
                                    ident[:P, :P])
                nc.scalar.copy(out=prs[:, r * P:(r + 1) * P],
                               in_=pstp[:BC, :P])
            nc.sync.dma_start(out=pers_d[:, :], in_=prs[:, :])

            fin = rw.tile([BC, P], F32, tag="fin")
            ftmp = rw.tile([BC, P], F32, tag="ftmp")
            for r in range(R):
                dst = fin if r == 0 else ftmp
                nc.vector.tensor_scalar_mul(out=dst[:, :],
                                            in0=prs[:, r * P:(r + 1) * P],
                                            scalar1=fwr[:, r:r + 1])
                if r > 0:
                    nc.vector.tensor_add(out=fin[:, :], in0=fin[:, :],
                                         in1=ftmp[:, :])
            nc.sync.dma_start(out=fin_d[:, :], in_=fin[:, :])

    nc.compile()
    return nc


_COMPILED = {}


def _get_compiled(R):
    if R not in _COMPILED:
        _COMPILED[R] = _build(R)
    return _COMPILED[R]


_BLK = np.zeros((RCH, 8, 32), np.float32)
for _p in range(RCH):
    if _p % WP < W:
        for _j in range(8):
            _BLK[_p, _j, 4 * _j + _p // WP] = 1.0


def _prep_x4(xs):
    x4 = np.zeros((4, BC, WP), np.float32)
    x4[0, :, 1:W] = xs[:, :W - 1]
    x4[1, :, :W] = xs
    x4[2, :, :W - 1] = xs[:, 1:]
    x4[3, :, :W] = 1.0
    return x4


def _prep_weights(inp, R):
    f = lambda a: np.ascontiguousarray(np.asarray(a, dtype=np.float32))
    conv1_w = f(inp["conv1_w"]); conv1_b = f(inp["conv1_b"])
    conv2_w = f(inp["conv2_w"]); conv2_b = f(inp["conv2_b"])
    w1b = np.zeros((4, H), np.float32)
    w1b[0:3] = conv1_w[:, 0, :].T
    w1b[3] = conv1_b
    w2f = conv2_w.transpose(1, 2, 0).reshape(2, 128, 3, H) \
        .transpose(1, 0, 2, 3).reshape(128, 6, H)
    gw = lambda a: np.asarray(a, np.float32)[:R].transpose(0, 2, 1) \
        .reshape(R, 2, 128, -1).transpose(0, 2, 1, 3)
    # fold msg linear into the GRU input weights (float64 for exactness):
    #   gi = Wih @ (Mw@h + mb) + bih = (Wih@Mw) @ h + (Wih@mb + bih)
    wih64 = np.asarray(inp["gru_wih"], np.float64)[:R]
    whh64 = np.asarray(inp["gru_whh"], np.float64)[:R]
    mw64 = np.asarray(inp["msg_w"], np.float64)[:R]
    mb64 = np.asarray(inp["msg_b"], np.float64)[:R]
    bih = np.asarray(inp["gru_bih"], np.float64)[:R]
    bhh = np.asarray(inp["gru_bhh"], np.float64)[:R]
    wihm = np.einsum("rgh,rhc->rgc", wih64, mw64)          # [R, 3H, H]
    bia = np.einsum("rgh,rh->rg", wih64, mb64) + bih       # [R, 3H]
    w0rz = (wihm[0] + whh64[0])[:2 * H].astype(np.float32)  # [2H, H]
    ch = lambda a: np.asarray(a, np.float32).reshape(R, 2, 128) \
        .transpose(0, 2, 1)
    ball = np.zeros((R, 128, 12), np.float32)
    ball[:, :, 0:4] = (bia + bhh)[:, :2 * H].astype(np.float32) \
        .reshape(R, 4, 128).transpose(0, 2, 1)
    ball[:, :, 4:6] = ch(bia[:, 2 * H:])
    ball[:, :, 6:8] = ch(bhh[:, 2 * H:])
    ball[:, :, 10] = f(inp["head_b1"])[:R]
    ball[:, :7, 11] = f(inp["head_b2"])[:R]
    ln2_g = f(inp["ln2_g"]); ln2_b = f(inp["ln2_b"])
    fusion_w1 = f(inp["fusion_w1"])
    m = {
        "w1b": w1b,
        "ln1g": f(inp["ln1_g"]),
        "b1bm": np.ascontiguousarray(
            np.where((np.arange(128) % WP < W)[:, None],
                     f(inp["ln1_b"])[None, :], 0.0).astype(np.float32)),
        "g2c": np.ascontiguousarray((ln2_g / W).reshape(2, 128).T),
        "b2c": np.ascontiguousarray(ln2_b.reshape(2, 128).T),
        "w2f": np.ascontiguousarray(w2f),
        "b2row": conv2_b[None, :],
        "wihmT": np.ascontiguousarray(gw(wihm.astype(np.float32))),
        "whhT": np.ascontiguousarray(gw(inp["gru_whh"])),
        "w0rzT": np.ascontiguousarray(
            w0rz.T.reshape(2, 128, 2 * H).transpose(1, 0, 2)),
        "hw1T": np.ascontiguousarray(gw(inp["head_w1"])),
        "hw2T": np.ascontiguousarray(f(inp["head_w2"])[:R].transpose(0, 2, 1)),
        "ball": ball,
        "fw1aT": np.ascontiguousarray(fusion_w1[:, :R * P].T),
        "fw1bT": np.ascontiguousarray(
            fusion_w1[:, RMAX * P:].T.reshape(2, 128, BC).transpose(1, 0, 2)),
        "fb1": f(inp["fusion_b1"])[:, None],
        "flng": f(inp["fusion_ln_g"]), "flnb": f(inp["fusion_ln_b"]),
        "fw2T": np.ascontiguousarray(f(inp["fusion_w2"]).T),
        "fb2": f(inp["fusion_b2"])[:, None],
        "blk": _BLK,
    }
    return m


def run_on_device(inputs, trace=False):
    """Shard, run the bass kernel on 8 cores, gather. Returns (outs, bkr)."""
    R = int(np.asarray(inputs["R"]))
    nc = _get_compiled(R)
    shared = _prep_weights(inputs, R)
    x = np.ascontiguousarray(np.asarray(inputs["x"], dtype=np.float32))
    in_maps = []
    for i in range(NCORES):
        mm = dict(shared)
        mm["x4"] = _prep_x4(x[i * BC:(i + 1) * BC])
        in_maps.append(mm)
    bkr = run_bass_kernel_spmd(nc, in_maps, core_ids=list(range(NCORES)),
                               trace=trace)
    res = bkr.results
    final = np.concatenate([res[i]["final"] for i in range(NCORES)], axis=0)
    pa = np.concatenate([res[i]["pers_act"] for i in range(NCORES)], axis=0)
    fw = np.concatenate([res[i]["fw"] for i in range(NCORES)], axis=0)
    pers = np.zeros((B, RMAX, P), np.float32)
    pers[:, :R, :] = pa.reshape(B, R, P)
    return (final.astype(np.float32), pers, fw.astype(np.float32)), bkr


def kernel(**inputs):
    outs, _ = run_on_device(inputs, trace=False)
    return outs


# revision 15
# speedup vs baseline: 3.0434x; 1.0788x over previous
"""Trainium2 Bass kernel for nn_DHCSTGCN (TCN encoder + GRU rounds + fusion).

Math note: in the reference, the confidence-modulation / attention block only
reaches the output through att.sum(-1), which is the sum of a softmax == 1
(up to fp32 rounding ~1e-6).  `messages` therefore equals the msg-linear
output `v` exactly, and the whole [B,N,H/2] block (and g/m/c/r_vec inputs)
drops out of the computation.

Sharding: data-parallel over batch, 64 rows per core on 8 cores; all weights
replicated.  Outputs are gathered/concatenated on host.

Layout: the TCN stage runs in a padded row space — each batch occupies 32
rows (30 time steps + 2 zero rows), so a 4-batch chunk is exactly 128 rows.
The zero columns double as conv SAME-padding when the transposed activations
are read with a +-1 shifted stride-1 slice, keeping every matmul stationary
operand a single free dimension.

Perf notes: big matmuls (moving dim 256) run as float32r (1 cycle/row vs 4
for fp32); the TCN is emitted phase-major so each engine's stream is dense;
LayerNorm gamma/beta of LN2 are folded into the h_current transpose-copy
(per-partition scale/bias on the ACT engine); LN1 gamma/beta and the
residual add run on the otherwise-idle GpSimd engine.
"""

import ml_dtypes
import numpy as np

import concourse.bacc as bacc
import concourse.bass as bass
import concourse.tile as tile
from concourse import mybir
from concourse.masks import make_identity
from concourse.bass_utils import run_bass_kernel_spmd

F32 = mybir.dt.float32
F32R = mybir.dt.float32r
BF16 = mybir.dt.float16
AF = mybir.ActivationFunctionType
OP = mybir.AluOpType
AX = mybir.AxisListType

B, N, H, W, P, RMAX = 512, 512, 256, 30, 7, 30
NCORES = 8
BC = B // NCORES      # 64 batch rows per core
WP = 32               # padded time steps per batch
CB = 4                # batches per row-chunk
RCH = CB * WP         # 128 padded rows per chunk
NCHUNK = BC // CB     # 16 chunks
H2 = H // 2           # 128
H3 = 3 * H            # 768
EPS = 1e-5


def _bcast(ap, parts):
    """Partition-broadcast a DRAM AP: [d...] -> [parts, d...] with step 0."""
    return bass.AP(tensor=ap.tensor, offset=ap.offset,
                   ap=[[0, parts]] + [list(d) for d in ap.ap])


def _r(ap):
    return ap.bitcast(F32R)


def _ln_core(nc, wk, src, dst, rows, eps_t):
    """y_hat = (src - mean)/sqrt(var+eps) over free dim (src clobbered)."""
    st = wk.tile([128, 6], F32, tag="lnst")
    nc.vector.bn_stats(out=st[:rows], in_=src)
    mv = wk.tile([128, 2], F32, tag="lnmv")
    nc.vector.bn_aggr(out=mv[:rows], in_=st[:rows])
    sd = wk.tile([128, 1], F32, tag="lnsd")
    nc.scalar.activation(out=sd[:rows], in_=mv[:rows, 1:2], func=AF.Sqrt,
                         bias=eps_t[:rows], scale=1.0)
    nc.vector.reciprocal(out=sd[:rows], in_=sd[:rows])
    nc.vector.tensor_scalar(out=dst, in0=src, scalar1=mv[:rows, 0:1],
                            scalar2=sd[:rows], op0=OP.subtract, op1=OP.mult)


def _build(R):
    nc = bacc.Bacc("TRN2", target_bir_lowering=False, debug=False)

    di = lambda name, shape: nc.dram_tensor(name, shape, F32, kind="ExternalInput")
    dir_ = lambda name, shape: nc.dram_tensor(name, shape, F32R, kind="ExternalInput")
    do = lambda name, shape: nc.dram_tensor(name, shape, F32, kind="ExternalOutput")

    x4_d = dir_("x4", [4, BC, WP])
    blk_d = dir_("blk", [RCH, 8, 32])
    w1b_d = dir_("w1b", [4, H])
    ln1g_d = di("ln1g", [H]); ln1b_d = di("b1bm", [128, H])
    g2c_d = di("g2c", [128, 2]); b2c_d = di("b2c", [128, 2])
    w2f_d = dir_("w2f", [128, 6, H])
    b2row_d = dir_("b2row", [1, H])
    dib = lambda name, shape: nc.dram_tensor(name, shape, BF16, kind="ExternalInput")
    wihm_d = dib("wihmT", [R, 128, 2, H3])
    whh_d = dib("whhT", [R, 128, 2, H3])
    w0rz_d = dib("w0rzT", [128, 2, 2 * H])
    hw1_d = di("hw1T", [R, 128, 2, H2])
    hw2_d = di("hw2T", [R, H2, P])
    ball_d = di("ball", [R, 128, 12])
    fw1a_d = di("fw1aT", [R * P, BC])
    fw1b_d = di("fw1bT", [128, 2, BC])
    fb1_d = di("fb1", [BC, 1])
    flng_d = di("flng", [BC]); flnb_d = di("flnb", [BC])
    fw2_d = di("fw2T", [BC, RMAX])
    fb2_d = di("fb2", [RMAX, 1])

    fin_d = do("final", [BC, P])
    pers_d = do("pers_act", [BC, R * P])
    fw_d = do("fw", [BC, RMAX])

    with tile.TileContext(nc) as tc:
        with (
            tc.tile_pool(name="singles", bufs=1) as sg,
            tc.tile_pool(name="wk", bufs=6) as wk,
            tc.tile_pool(name="wp", bufs=max(2, R)) as wp,
            tc.tile_pool(name="rw", bufs=3) as rw,
            tc.tile_pool(name="psA", bufs=2, space="PSUM") as psA,
            tc.tile_pool(name="psB", bufs=3, space="PSUM") as psB,
            tc.tile_pool(name="psT", bufs=2, space="PSUM") as psT,
            tc.tile_pool(name="psM", bufs=1, space="PSUM") as psM,
        ):
            # ---------- critical-path inputs first ----------
            t4 = sg.tile([4, BC * WP], F32R, tag="t4")
            nc.sync.dma_start(out=t4[:, :],
                              in_=x4_d[:, :, :].rearrange("k b w -> k (b w)"))
            w1b_t = sg.tile([4, H], F32R, tag="w1b")
            nc.sync.dma_start(out=w1b_t[:, :], in_=w1b_d[:, :])
            g1b = sg.tile([128, H], F32, tag="g1b")
            nc.sync.dma_start(out=g1b[:, :], in_=_bcast(ln1g_d[:], 128))
            b1b = sg.tile([128, H], F32, tag="b1b")
            nc.sync.dma_start(out=b1b[:, :], in_=ln1b_d[:, :])
            w2f_t = sg.tile([128, 6, H], F32R, tag="w2f")
            nc.sync.dma_start(out=w2f_t[:, 0:3, :], in_=w2f_d[:, 0:3, :])
            nc.sync.dma_start(out=w2f_t[:, 3:6, :], in_=w2f_d[:, 3:6, :])
            b2row_t = sg.tile([1, H], F32R, tag="b2row")
            nc.sync.dma_start(out=b2row_t[:, :], in_=b2row_d[:, :])
            g2c = sg.tile([128, 2], F32, tag="g2c")
            nc.sync.dma_start(out=g2c[:, :], in_=g2c_d[:, :])
            b2c = sg.tile([128, 2], F32, tag="b2c")
            nc.sync.dma_start(out=b2c[:, :], in_=b2c_d[:, :])
            blk = sg.tile([RCH, 8, 32], F32R, tag="blk")
            nc.sync.dma_start(out=blk[:, :, :], in_=blk_d[:, :, :])

            ident = sg.tile([128, 128], F32, tag="ident")
            make_identity(nc, ident[:, :])
            eps_t = sg.tile([128, 1], F32, tag="eps")
            nc.vector.memset(eps_t[:, :], EPS)
            ones_f = sg.tile([1, 128], F32, tag="ones_f")
            nc.vector.memset(ones_f[:, :], 1.0)
            ones1 = sg.tile([1, 128], F32R, tag="ones1")
            nc.vector.tensor_copy(out=ones1[:, :], in_=ones_f[:, :])
            zerot = sg.tile([128, 2], F32, tag="zerot")
            nc.vector.memset(zerot[:, :], 0.0)

            flngb = sg.tile([BC, BC], F32, tag="flngb")
            nc.sync.dma_start(out=flngb[:, :], in_=_bcast(flng_d[:], BC))
            flnbb = sg.tile([BC, BC], F32, tag="flnbb")
            nc.sync.dma_start(out=flnbb[:, :], in_=_bcast(flnb_d[:], BC))
            fb1_t = sg.tile([BC, 1], F32, tag="fb1")
            nc.sync.dma_start(out=fb1_t[:, :], in_=fb1_d[:, :])
            fw2_t = sg.tile([BC, RMAX], F32, tag="fw2")
            nc.sync.dma_start(out=fw2_t[:, :], in_=fw2_d[:, :])
            fb2_t = sg.tile([RMAX, 1], F32, tag="fb2")
            nc.sync.dma_start(out=fb2_t[:, :], in_=fb2_d[:, :])
            fw1b_t = sg.tile([128, 2, BC], F32, tag="fw1b")
            nc.sync.dma_start(out=fw1b_t[:, :, :], in_=fw1b_d[:, :, :])
            fw1a_t = []
            for r in range(R):
                t = sg.tile([P, BC], F32, tag=f"fw1a{r}", name=f"fw1a{r}")
                nc.sync.dma_start(out=t[:, :], in_=fw1a_d[r * P:(r + 1) * P, :])
                fw1a_t.append(t)

            h_cur = sg.tile([BC, H], F32, tag="h_cur")

            # per-chunk persistents
            r2s = [sg.tile([RCH, H], F32, tag=f"r2_{j}", name=f"r2_{j}")
                   for j in range(NCHUNK)]
            yTs = [sg.tile([128, 2, RCH + 2], F32R, tag=f"yT_{j}", name=f"yT_{j}")
                   for j in range(NCHUNK)]

            # ---------- TCN L1: conv1 + LN1 + gamma/beta (gpsimd) ----------
            for j in range(NCHUNK):
                ps1 = psA.tile([RCH, H], F32, tag="ps1")
                nc.tensor.matmul(ps1[:, :], t4[:, j * RCH:(j + 1) * RCH],
                                 w1b_t[:, :], start=True, stop=True)
                y = wk.tile([RCH, H], F32, tag="y")
                nc.scalar.activation(out=y[:, :], in_=ps1[:, :], func=AF.Relu)
                yh = wk.tile([RCH, H], F32, tag="yh")
                _ln_core(nc, wk, y[:, :], yh[:, :], RCH, eps_t)
                # r2 = yh*g1 + b1 (true LN1 output) on GpSimd
                nc.gpsimd.tensor_mul(out=r2s[j][:, :], in0=yh[:, :],
                                     in1=g1b[:, :])
                eng = nc.vector if j % 2 == 0 else nc.gpsimd
                eng.tensor_add(out=r2s[j][:, :], in0=r2s[j][:, :],
                               in1=b1b[:, :])

            # ---------- TCN L2: transpose LN1 out into padded col space ----
            for j in range(NCHUNK):
                yT = yTs[j]
                nc.vector.tensor_copy(
                    out=yT[:, :, 0:1],
                    in_=zerot[:, :].rearrange("p (a c) -> p a c", c=1))
                nc.vector.tensor_copy(
                    out=yT[:, :, RCH + 1:RCH + 2],
                    in_=zerot[:, :].rearrange("p (a c) -> p a c", c=1))
                for cc in range(2):
                    pst = psT.tile([128, 128], F32, tag="pst")
                    nc.tensor.transpose(pst[:, :],
                                        r2s[j][:, cc * 128:(cc + 1) * 128],
                                        ident[:, :])
                    # pad rows of r2 are exactly zero (masked beta), so the
                    # full-block copy leaves conv SAME-padding zeros in place
                    nc.scalar.copy(out=yT[:, cc, 1:RCH + 1], in_=pst[:, :])

            # ---------- TCN L3: conv2 + post (lagged) + mean-w ----------
            def post(j):
                t2 = wk.tile([RCH, H], F32, tag="t2")
                nc.scalar.activation(out=t2[:, :], in_=ps2s[j][:, :],
                                     func=AF.Relu)
                y2 = wk.tile([RCH, H], F32, tag="y2")
                nc.gpsimd.tensor_add(out=y2[:, :], in0=t2[:, :],
                                     in1=r2s[j][:, :])
                y2n = wk.tile([RCH, H], F32R, tag="y2n")
                _ln_core(nc, wk, y2[:, :], y2n[:, :], RCH, eps_t)
                gi, jj = divmod(j, 8)
                if jj == 0:
                    psm32s[gi] = psM.tile([32, H], F32, tag="psm",
                                          name=f"psm32_{gi}")
                nc.tensor.matmul(psm32s[gi][:, :], blk[:, jj, :],
                                 y2n[:, :], start=(jj == 0),
                                 stop=(jj == 7))
                if jj == 7:
                    nc.scalar.copy(out=h_cur[gi * 32:(gi + 1) * 32, :],
                                   in_=psm32s[gi][:, :])

            ps2s = {}
            psm32s = {}
            for j in range(NCHUNK):
                ps2 = psB.tile([RCH, H], F32, tag="ps2")
                ps2s[j] = ps2
                kk = 0
                for cc in range(2):
                    for dw in range(3):
                        nc.tensor.matmul(ps2[:, :],
                                         yTs[j][:, cc, dw:dw + RCH],
                                         w2f_t[:, cc * 3 + dw, :],
                                         start=(kk == 0), stop=False)
                        kk += 1
                nc.tensor.matmul(ps2[:, :], ones1[:, :], b2row_t[:, :],
                                 start=False, stop=True)
                if j >= 2:
                    post(j - 2)
                    del ps2s[j - 2]
            post(NCHUNK - 2)
            post(NCHUNK - 1)

            # ---------- h_current transposed: hT[c, cc, b] (LN2 g/b folded)
            hT = sg.tile([128, 2, BC], F32, tag="hT")
            for cc in range(2):
                pst = psT.tile([128, 128], F32, tag="pst")
                nc.tensor.transpose(pst[:, :BC],
                                    h_cur[:, cc * 128:(cc + 1) * 128],
                                    ident[:BC, :BC])
                nc.scalar.activation(out=hT[:, cc, :], in_=pst[:, :BC],
                                     func=AF.Identity,
                                     scale=g2c[:, cc:cc + 1],
                                     bias=b2c[:, cc:cc + 1])

            pred_t = [sg.tile([P, BC], F32, tag=f"pred{r}", name=f"pred{r}")
                      for r in range(R)]
            hTb = sg.tile([128, 2, BC], BF16, tag="hTb")
            nc.vector.tensor_copy(out=hTb[:, :, :], in_=hT[:, :, :])

            # ---------- rounds ----------
            # gi_r = Wih_r@(Mw_r@h_cur + mb_r) + bih_r is host-folded to
            # wihm_r@h_cur + bia_r; round 0 merges wihm_0+whh_0 for r/z
            # (h_state0 == h_current); later rounds hoist their gi parts.
            hp2 = [hT[:, 0, :], hT[:, 1, :]]
            hp2b = None  # set after hTb exists

            wihms, whhs, hw1s, hw2s, balls = [], [], [], [], []
            for r in range(R):
                wihm = wp.tile([128, 2, H3], BF16, tag="wihm")
                for q in range(4):
                    s0, s1 = q * (H3 // 2), (q + 1) * (H3 // 2)
                    cc, lo, hi = (0, s0, s1) if q < 2 else (1, s0 - H3, s1 - H3)
                    nc.sync.dma_start(out=wihm[:, cc, lo:hi],
                                      in_=wihm_d[r, :, cc, lo:hi])
                whh = wp.tile([128, 2, H3], BF16, tag="whh")
                for q in range(4):
                    s0, s1 = q * (H3 // 2), (q + 1) * (H3 // 2)
                    cc, lo, hi = (0, s0, s1) if q < 2 else (1, s0 - H3, s1 - H3)
                    nc.sync.dma_start(out=whh[:, cc, lo:hi],
                                      in_=whh_d[r, :, cc, lo:hi])
                hw1 = wp.tile([128, 2, H2], F32, tag="hw1")
                nc.sync.dma_start(out=hw1[:, :, :], in_=hw1_d[r, :, :, :])
                hw2 = wp.tile([H2, P], F32, tag="hw2")
                nc.sync.dma_start(out=hw2[:, :], in_=hw2_d[r, :, :])
                ball = wp.tile([128, 12], F32, tag="ball")
                nc.sync.dma_start(out=ball[:, :], in_=ball_d[r, :, :])
                wihms.append(wihm); whhs.append(whh)
                hw1s.append(hw1); hw2s.append(hw2); balls.append(ball)
            w0rz = wp.tile([128, 2, 2 * H], BF16, tag="w0rz")
            nc.sync.dma_start(out=w0rz[:, :, :], in_=w0rz_d[:, :, :])

            hp2b = [hTb[:, 0, :], hTb[:, 1, :]]
            # hoisted gi parts for rounds >= 1 (depend only on h_current)
            gia = {}
            for r in range(1, R):
                for m in range(6):
                    pgi = psB.tile([RCH, H], F32, tag="ps2")
                    for cc in range(2):
                        nc.tensor.matmul(pgi[:, :BC],
                                         wihms[r][:, cc,
                                                  m * 128:(m + 1) * 128],
                                         hTb[:, cc, :],
                                         start=(cc == 0), stop=(cc == 1))
                    bcol = m if m < 4 else 4 + (m - 4)
                    gt = rw.tile([128, BC], F32, tag=f"gia{m}")
                    nc.scalar.activation(out=gt[:, :], in_=pgi[:, :BC],
                                         func=AF.Identity,
                                         bias=balls[r][:, bcol:bcol + 1])
                    gia[(r, m)] = gt

            for r in range(R):
                ball = balls[r]
                brz = ball[:, 0:4]
                binb = ball[:, 4:6]
                bhnb = ball[:, 6:8]
                hb1 = ball[:, 10:11]
                hb2 = ball[:7, 11:12]

                # r/z gates
                gates = []
                for m in range(4):
                    pg = psA.tile([128, BC], F32, tag="ps1")
                    if r == 0:
                        for cc in range(2):
                            nc.tensor.matmul(pg[:, :],
                                             w0rz[:, cc,
                                                  m * 128:(m + 1) * 128],
                                             hp2b[cc],
                                             start=(cc == 0), stop=(cc == 1))
                        g = rw.tile([128, BC], F32, tag=f"g{m}")
                        nc.scalar.activation(out=g[:, :], in_=pg[:, :],
                                             func=AF.Sigmoid,
                                             bias=brz[:, m:m + 1])
                    else:
                        for cc in range(2):
                            nc.tensor.matmul(pg[:, :],
                                             whhs[r][:, cc,
                                                     m * 128:(m + 1) * 128],
                                             hp2b[cc],
                                             start=(cc == 0), stop=(cc == 1))
                        gp = rw.tile([128, BC], F32, tag=f"gp{m}")
                        nc.vector.tensor_add(out=gp[:, :], in0=pg[:, :],
                                             in1=gia[(r, m)][:, :])
                        g = rw.tile([128, BC], F32, tag=f"g{m}")
                        nc.scalar.activation(out=g[:, :], in_=gp[:, :],
                                             func=AF.Sigmoid)
                    gates.append(g)

                # n gate: tanh(gi_n + rg * (Whh_n@h + bhn))
                ns = []
                for mi in range(2):
                    m = 4 + mi
                    ph = psT.tile([128, 128], F32, tag="pst")
                    for cc in range(2):
                        nc.tensor.matmul(ph[:, :BC],
                                         whhs[r][:, cc,
                                                 m * 128:(m + 1) * 128],
                                         hp2b[cc],
                                         start=(cc == 0), stop=(cc == 1))
                    hn = rw.tile([128, BC], F32, tag=f"hn{mi}")
                    nc.scalar.activation(out=hn[:, :], in_=ph[:, :BC],
                                         func=AF.Identity,
                                         bias=bhnb[:, mi:mi + 1])
                    nc.vector.tensor_mul(out=hn[:, :], in0=gates[mi][:, :],
                                         in1=hn[:, :])
                    if r == 0:
                        pi = psA.tile([128, BC], F32, tag="ps1")
                        for cc in range(2):
                            nc.tensor.matmul(pi[:, :],
                                             wihms[0][:, cc,
                                                      m * 128:(m + 1) * 128],
                                             hp2b[cc],
                                             start=(cc == 0), stop=(cc == 1))
                        nc.vector.tensor_add(out=hn[:, :], in0=hn[:, :],
                                             in1=pi[:, :])
                        n_t = rw.tile([128, BC], F32, tag=f"n{mi}")
                        nc.scalar.activation(out=n_t[:, :], in_=hn[:, :],
                                             func=AF.Tanh,
                                             bias=binb[:, mi:mi + 1])
                    else:
                        nc.vector.tensor_add(out=hn[:, :], in0=hn[:, :],
                                             in1=gia[(r, m)][:, :])
                        n_t = rw.tile([128, BC], F32, tag=f"n{mi}")
                        nc.scalar.activation(out=n_t[:, :], in_=hn[:, :],
                                             func=AF.Tanh, bias=0.0)
                    ns.append(n_t)

                # h' = n + z*(h - n)
                hnew = rw.tile([128, 2, BC], F32, tag="hstate")
                for mi in range(2):
                    d = rw.tile([128, BC], F32, tag=f"d{mi}")
                    nc.vector.tensor_sub(out=d[:, :], in0=hp2[mi],
                                         in1=ns[mi][:, :])
                    nc.vector.tensor_mul(out=d[:, :], in0=gates[2 + mi][:, :],
                                         in1=d[:, :])
                    nc.vector.tensor_add(out=hnew[:, mi, :], in0=ns[mi][:, :],
                                         in1=d[:, :])
                hp2 = [hnew[:, 0, :], hnew[:, 1, :]]
                hnb = rw.tile([128, 2, BC], BF16, tag="hstateb")
                nc.vector.tensor_copy(out=hnb[:, :, :], in_=hnew[:, :, :])
                hp2b = [hnb[:, 0, :], hnb[:, 1, :]]

                # head: pred = head_w2 @ relu(head_w1 @ h' + b1) + b2
                pp = psB.tile([RCH, H], F32, tag="ps2")
                for cc in range(2):
                    nc.tensor.matmul(pp[:H2, :BC], hw1s[r][:, cc, :], hp2[cc],
                                     start=(cc == 0), stop=(cc == 1))
                p1 = rw.tile([H2, BC], F32, tag="p1")
                nc.scalar.activation(out=p1[:, :], in_=pp[:H2, :BC],
                                     func=AF.Relu, bias=hb1[:H2, 0:1])
                pq = psB.tile([RCH, H], F32, tag="ps2")
                nc.tensor.matmul(pq[:P, :BC], hw2s[r][:, :], p1[:, :],
                                 start=True, stop=True)
                nc.scalar.activation(out=pred_t[r][:, :], in_=pq[:P, :BC],
                                     func=AF.Identity, bias=hb2[:, 0:1])

            # ---------- fusion ----------
            phf = psA.tile([128, BC], F32, tag="ps1")
            nk = R + 2
            kk = 0
            for r in range(R):
                nc.tensor.matmul(phf[:BC, :], fw1a_t[r][:, :],
                                 pred_t[r][:, :],
                                 start=(kk == 0), stop=False)
                kk += 1
            for cc in range(2):
                nc.tensor.matmul(phf[:BC, :], fw1b_t[:, cc, :], hp2[cc],
                                 start=(kk == 0), stop=(kk == nk - 1))
                kk += 1
            hfT = rw.tile([BC, BC], F32, tag="hfT")
            nc.scalar.activation(out=hfT[:, :], in_=phf[:BC, :],
                                 func=AF.Relu, bias=fb1_t[:, 0:1])
            pstf = psT.tile([128, 128], F32, tag="pst")
            nc.tensor.transpose(pstf[:BC, :BC], hfT[:, :], ident[:BC, :BC])
            hfr = rw.tile([BC, BC], F32, tag="hfr")
            nc.scalar.copy(out=hfr[:, :], in_=pstf[:BC, :BC])
            hfn0 = rw.tile([BC, BC], F32, tag="hfn0")
            _ln_core(nc, wk, hfr[:, :], hfn0[:, :], BC, eps_t)
            hfn = rw.tile([BC, BC], F32, tag="hfn")
            nc.vector.tensor_mul(out=hfn0[:, :], in0=hfn0[:, :],
                                 in1=flngb[:, :])
            nc.vector.tensor_add(out=hfn[:, :], in0=hfn0[:, :],
                                 in1=flnbb[:, :])
            pstg = psT.tile([128, 128], F32, tag="pst")
            nc.tensor.transpose(pstg[:BC, :BC], hfn[:, :], ident[:BC, :BC])
            hfnT = rw.tile([BC, BC], F32, tag="hfnT")
            nc.scalar.copy(out=hfnT[:, :], in_=pstg[:BC, :BC])

            plg = psA.tile([128, BC], F32, tag="ps1")
            nc.tensor.matmul(plg[:RMAX, :], fw2_t[:, :], hfnT[:, :],
                             start=True, stop=True)
            lgT = rw.tile([RMAX, BC], F32, tag="lgT")
            nc.scalar.activation(out=lgT[:, :], in_=plg[:RMAX, :],
                                 func=AF.Identity, bias=fb2_t[:, 0:1])
            psth = psT.tile([128, 128], F32, tag="pst")
            nc.tensor.transpose(psth[:BC, :RMAX], lgT[:, :],
                                ident[:RMAX, :RMAX])
            lg = rw.tile([BC, RMAX], F32, tag="lg")
            nc.scalar.copy(out=lg[:, :], in_=psth[:BC, :RMAX])

            mx = rw.tile([BC, 1], F32, tag="mx")
            nc.vector.reduce_max(out=mx[:, :], in_=lg[:, :], axis=AX.X)
            nc.vector.tensor_scalar(out=lg[:, :], in0=lg[:, :],
                                    scalar1=mx[:, 0:1], scalar2=None,
                                    op0=OP.subtract)
            ex = rw.tile([BC, RMAX], F32, tag="ex")
            sm = rw.tile([BC, 1], F32, tag="sm")
            nc.scalar.activation(out=ex[:, :], in_=lg[:, :], func=AF.Exp,
                                 accum_out=sm[:, 0:1])
            nc.vector.reciprocal(out=sm[:, :], in_=sm[:, :])
            fwr = rw.tile([BC, RMAX], F32, tag="fwr")
            nc.vector.tensor_scalar_mul(out=fwr[:, :], in0=ex[:, :],
                                        scalar1=sm[:, 0:1])
            nc.sync.dma_start(out=fw_d[:, :], in_=fwr[:, :])

            prs = rw.tile([BC, R * P], F32, tag="prs")
            for r in range(R):
                pstp = psT.tile([128, 128], F32, tag="pst")
                nc.tensor.transpose(pstp[:BC, :P], pred_t[r][:, :],
                                    ident[:P, :P])
                nc.scalar.copy(out=prs[:, r * P:(r + 1) * P],
                               in_=pstp[:BC, :P])
            nc.sync.dma_start(out=pers_d[:, :], in_=prs[:, :])

            fin = rw.tile([BC, P], F32, tag="fin")
            ftmp = rw.tile([BC, P], F32, tag="ftmp")
            for r in range(R):
                dst = fin if r == 0 else ftmp
                nc.vector.tensor_scalar_mul(out=dst[:, :],
                                            in0=prs[:, r * P:(r + 1) * P],
                                            scalar1=fwr[:, r:r + 1])
                if r > 0:
                    nc.vector.tensor_add(out=fin[:, :], in0=fin[:, :],
                                         in1=ftmp[:, :])
            nc.sync.dma_start(out=fin_d[:, :], in_=fin[:, :])

    nc.compile()
    return nc


_COMPILED = {}


def _get_compiled(R):
    if R not in _COMPILED:
        _COMPILED[R] = _build(R)
    return _COMPILED[R]


_BLK = np.zeros((RCH, 8, 32), np.float32)
for _p in range(RCH):
    if _p % WP < W:
        for _j in range(8):
            _BLK[_p, _j, 4 * _j + _p // WP] = 1.0


def _prep_x4(xs):
    x4 = np.zeros((4, BC, WP), np.float32)
    x4[0, :, 1:W] = xs[:, :W - 1]
    x4[1, :, :W] = xs
    x4[2, :, :W - 1] = xs[:, 1:]
    x4[3, :, :W] = 1.0
    return x4


def _prep_weights(inp, R):
    f = lambda a: np.ascontiguousarray(np.asarray(a, dtype=np.float32))
    conv1_w = f(inp["conv1_w"]); conv1_b = f(inp["conv1_b"])
    conv2_w = f(inp["conv2_w"]); conv2_b = f(inp["conv2_b"])
    w1b = np.zeros((4, H), np.float32)
    w1b[0:3] = conv1_w[:, 0, :].T
    w1b[3] = conv1_b
    w2f = conv2_w.transpose(1, 2, 0).reshape(2, 128, 3, H) \
        .transpose(1, 0, 2, 3).reshape(128, 6, H)
    gw = lambda a: np.asarray(a, np.float32)[:R].transpose(0, 2, 1) \
        .reshape(R, 2, 128, -1).transpose(0, 2, 1, 3)
    # fold msg linear into the GRU input weights (float64 for exactness):
    #   gi = Wih @ (Mw@h + mb) + bih = (Wih@Mw) @ h + (Wih@mb + bih)
    wih64 = np.asarray(inp["gru_wih"], np.float64)[:R]
    whh64 = np.asarray(inp["gru_whh"], np.float64)[:R]
    mw64 = np.asarray(inp["msg_w"], np.float64)[:R]
    mb64 = np.asarray(inp["msg_b"], np.float64)[:R]
    bih = np.asarray(inp["gru_bih"], np.float64)[:R]
    bhh = np.asarray(inp["gru_bhh"], np.float64)[:R]
    wihm = np.einsum("rgh,rhc->rgc", wih64, mw64)          # [R, 3H, H]
    bia = np.einsum("rgh,rh->rg", wih64, mb64) + bih       # [R, 3H]
    w0rz = (wihm[0] + whh64[0])[:2 * H].astype(np.float32)  # [2H, H]
    ch = lambda a: np.asarray(a, np.float32).reshape(R, 2, 128) \
        .transpose(0, 2, 1)
    ball = np.zeros((R, 128, 12), np.float32)
    ball[:, :, 0:4] = (bia + bhh)[:, :2 * H].astype(np.float32) \
        .reshape(R, 4, 128).transpose(0, 2, 1)
    ball[:, :, 4:6] = ch(bia[:, 2 * H:])
    ball[:, :, 6:8] = ch(bhh[:, 2 * H:])
    ball[:, :, 10] = f(inp["head_b1"])[:R]
    ball[:, :7, 11] = f(inp["head_b2"])[:R]
    ln2_g = f(inp["ln2_g"]); ln2_b = f(inp["ln2_b"])
    fusion_w1 = f(inp["fusion_w1"])
    m = {
        "w1b": w1b,
        "ln1g": f(inp["ln1_g"]),
        "b1bm": np.ascontiguousarray(
            np.where((np.arange(128) % WP < W)[:, None],
                     f(inp["ln1_b"])[None, :], 0.0).astype(np.float32)),
        "g2c": np.ascontiguousarray((ln2_g / W).reshape(2, 128).T),
        "b2c": np.ascontiguousarray(ln2_b.reshape(2, 128).T),
        "w2f": np.ascontiguousarray(w2f),
        "b2row": conv2_b[None, :],
        "wihmT": np.ascontiguousarray(gw(wihm.astype(np.float32)).astype(np.float16)),
        "whhT": np.ascontiguousarray(gw(inp["gru_whh"]).astype(np.float16)),
        "w0rzT": np.ascontiguousarray(
            w0rz.T.reshape(2, 128, 2 * H).transpose(1, 0, 2)
            .astype(np.float16)),
        "hw1T": np.ascontiguousarray(gw(inp["head_w1"])),
        "hw2T": np.ascontiguousarray(f(inp["head_w2"])[:R].transpose(0, 2, 1)),
        "ball": ball,
        "fw1aT": np.ascontiguousarray(fusion_w1[:, :R * P].T),
        "fw1bT": np.ascontiguousarray(
            fusion_w1[:, RMAX * P:].T.reshape(2, 128, BC).transpose(1, 0, 2)),
        "fb1": f(inp["fusion_b1"])[:, None],
        "flng": f(inp["fusion_ln_g"]), "flnb": f(inp["fusion_ln_b"]),
        "fw2T": np.ascontiguousarray(f(inp["fusion_w2"]).T),
        "fb2": f(inp["fusion_b2"])[:, None],
        "blk": _BLK,
    }
    return m


def run_on_device(inputs, trace=False):
    """Shard, run the bass kernel on 8 cores, gather. Returns (outs, bkr)."""
    R = int(np.asarray(inputs["R"]))
    nc = _get_compiled(R)
    shared = _prep_weights(inputs, R)
    x = np.ascontiguousarray(np.asarray(inputs["x"], dtype=np.float32))
    in_maps = []
    for i in range(NCORES):
        mm = dict(shared)
        mm["x4"] = _prep_x4(x[i * BC:(i + 1) * BC])
        in_maps.append(mm)
    bkr = run_bass_kernel_spmd(nc, in_maps, core_ids=list(range(NCORES)),
                               trace=trace)
    res = bkr.results
    final = np.concatenate([res[i]["final"] for i in range(NCORES)], axis=0)
    pa = np.concatenate([res[i]["pers_act"] for i in range(NCORES)], axis=0)
    fw = np.concatenate([res[i]["fw"] for i in range(NCORES)], axis=0)
    pers = np.zeros((B, RMAX, P), np.float32)
    pers[:, :R, :] = pa.reshape(B, R, P)
    return (final.astype(np.float32), pers, fw.astype(np.float32)), bkr


def kernel(**inputs):
    outs, _ = run_on_device(inputs, trace=False)
    return outs
